# revision 22
# baseline (speedup 1.0000x reference)
"""NeuronMemory retrieval kernel v4 for 8 TRN2 NeuronCores.

Data-parallel over tokens (512/core, 4 tiles of 128). Per-core, per tile:
  A:  router scores (PE) + softmax -> wts f32 [128,16]
  B:  y = x @ W_n for 16 neurons (PE, f16, 32 matmuls) -> y16 f16 SBUF
      QT[r,tok] = sum_n diag(w_n)-weighted y via 16 PE matmuls (diag as rhs)
      qt16 = f16(QT * SCALE); q16 = transpose(qt16) via PE
  C:  16 windows of 2048 keys: scores = qt16^T @ KT16 (PE, f16) -> PSUM f32
      L1 pair-reduce: sp1[w*1024+j] = max(s[2048w+j], s[2048w+1024+j])
        plan A (Z_A windows): scalar stages whole window f16, DVE TT 2x
        plan B (rest):        scalar stages 2nd half,     DVE TT(PSUM,SBUF)
  tree: sp1 [128,16384] -> blockmax bm [128,256] (64 pairs/block) on DVE
  spill: sp1 -> DRAM rows [128*256, 64] f16 (one 4MB DMA)
  L2: top-8 blocks/token (max8+fi8 on bm)
  G1: gather 8 winning block-rows (64 pair-values each) -> g1 [128,512]
  L3: top-8 pairs/token (max8+fi8 on g1) -> pairidx[8] (global pair ids)
  G2: gather 8 K-pair rows (256 f16) from KP16 -> g2; rescore 16 cands
      exactly on DVE (TT mult + TR add vs q16) -> rsc f32 -> f16
  top8: max8+fi8 on rsc16 -> 8 winners; member bit + key reconstruction;
      softmax(v8) -> w8 f32
  G3: gather 8 V rows (2KB f16) -> g3; Vsum via 8 diag matmuls on PE -> out

Pair rows: KP16[w*1024+j] = (K[2048w+j], K[2048w+1024+j]), both f16.
key = pairidx + 1024*(pairidx>>10) + 1024*member.
"""
import copy as _copy

import numpy as np

import concourse.bacc as bacc
import concourse.bass as bass
import concourse.mybir as mybir
from concourse.tile import TileContext
from concourse.bass_utils import run_bass_kernel_spmd

P = 128
D_MODEL = 1024
RANK = 128
N_COMPRESS = 16
N_KNOWLEDGE = 32768
K_TOP = 8
B, S = 2, 2048
N_CORES = 8
TOK_PER_CORE = (B * S) // N_CORES      # 512
N_TILES = TOK_PER_CORE // P            # 4
N_DC = D_MODEL // P                    # 8
N_WIN = 16                             # score windows per tile
WIN = N_KNOWLEDGE // N_WIN             # 2048 keys per window
HALF = WIN // 2                        # 1024 pairs per window
N_PAIR = N_KNOWLEDGE // 2              # 16384 pairs per tile
BLK = 64                               # pairs per block
N_BLK = N_PAIR // BLK                  # 256 blocks
NBSEL = 9                              # blocks gathered per token (8 + tie slack)
NPSEL = 10                             # pairs rescored per token (8 + tie slack)
NCAND = 2 * NPSEL                      # candidate keys
Z_A = 6                                # plan-A windows per tile (0..16)
SCALE = 1.0 / np.sqrt(np.float32(RANK))
SSEL = 0.5                             # extra selection scale (folded into qt16/q16)

f32 = mybir.dt.float32
f16 = mybir.dt.float16
u32 = mybir.dt.uint32

AL = mybir.AluOpType


def _build(dbg=False):
    nc = bacc.Bacc("TRN2", target_bir_lowering=False, debug=False, num_devices=N_CORES)

    xT = nc.declare_dram_parameter("xT", [P, N_DC * TOK_PER_CORE], f32, isOutput=False)
    rw = nc.declare_dram_parameter("rw", [P, N_DC * N_COMPRESS], f32, isOutput=False)
    # Wg[p, (gh*8+dc)*1024 + ln*128 + r] = cn[8*gh+ln, dc*128+p, r]; streamed
    Wg = nc.declare_dram_parameter("Wg", [P, 2 * N_DC * 1024], f32, isOutput=False)
    KT16 = nc.declare_dram_parameter("KT16", [P, N_KNOWLEDGE], f16, isOutput=False)
    KP32 = nc.declare_dram_parameter("KP32", [N_PAIR, 2 * RANK], f32, isOutput=False)
    VD16 = nc.declare_dram_parameter("VD16", [N_KNOWLEDGE, D_MODEL], f16, isOutput=False)
    ident = nc.declare_dram_parameter("ident", [P, P], f16, isOutput=False)
    idf32 = nc.declare_dram_parameter("idf32", [P, P], f32, isOutput=False)
    iotaP = nc.declare_dram_parameter("iotaP", [P, 1], f32, isOutput=False)
    io16 = nc.declare_dram_parameter("io16", [P, 24], f32, isOutput=False)
    out = nc.declare_dram_parameter("out", [TOK_PER_CORE, D_MODEL], f32, isOutput=True)
    if dbg:
        d_wts = nc.declare_dram_parameter("d_wts", [P, N_TILES * N_COMPRESS], f32, isOutput=True)
        d_qt = nc.declare_dram_parameter("d_qt", [P, N_TILES * P], f16, isOutput=True)
        d_bm = nc.declare_dram_parameter("d_bm", [P, N_TILES * N_BLK], f16, isOutput=True)
        d_g1 = nc.declare_dram_parameter("d_g1", [P, N_TILES * 512], f16, isOutput=True)
        d_pj = nc.declare_dram_parameter("d_pj", [P, N_TILES * NPSEL], f32, isOutput=True)
        d_rs = nc.declare_dram_parameter("d_rs", [P, N_TILES * NCAND], f32, isOutput=True)
        d_key = nc.declare_dram_parameter("d_key", [P, N_TILES * 8], f32, isOutput=True)
        d_w8 = nc.declare_dram_parameter("d_w8", [P, N_TILES * 8], f32, isOutput=True)

    sp1D = {t: nc.dram_tensor(f"sp1D_{t}", [P * N_BLK, BLK], f16) for t in range(N_TILES)}

    with TileContext(nc) as tc:
        with (
            tc.tile_pool(name="const", bufs=1) as cpool,
            tc.tile_pool(name="sp1p", bufs=1) as sp1pool,
            tc.tile_pool(name="stg", bufs=2) as stgpool,
            tc.tile_pool(name="wgs", bufs=2) as wgpool,
            tc.tile_pool(name="dgp", bufs=2) as dgpool,
            tc.tile_pool(name="qp", bufs=1) as qpool,
            tc.tile_pool(name="g1p", bufs=2) as g1pool,
            tc.tile_pool(name="g2p", bufs=1) as g2pool,
            tc.tile_pool(name="g3p", bufs=1) as g3pool,
            tc.tile_pool(name="pr", bufs=1) as prpool,
            tc.tile_pool(name="acc", bufs=2) as apool,
            tc.tile_pool(name="sm", bufs=2) as spool,
            tc.tile_pool(name="psw", bufs=2, space="PSUM") as psw,
        ):
            # ---------------- persistent loads ----------------
            xT_sb = cpool.tile([P, N_DC * TOK_PER_CORE], f32)
            rw_sb = cpool.tile([P, N_DC * N_COMPRESS], f32)
            kt_sb = cpool.tile([P, N_KNOWLEDGE], f16)
            id_sb = cpool.tile([P, P], f16)
            idf_sb = cpool.tile([P, P], f32)
            iota_sb = cpool.tile([P, 1], f32)
            io16_sb = cpool.tile([P, 24], f32)
            nc.sync.dma_start(out=xT_sb[:], in_=xT[:])
            nc.sync.dma_start(out=rw_sb[:], in_=rw[:])
            nc.sync.dma_start(out=idf_sb[:], in_=idf32[:])
            for q in range(4):
                nc.sync.dma_start(out=kt_sb[:, q * 8192:(q + 1) * 8192],
                                  in_=KT16[:, q * 8192:(q + 1) * 8192])
            nc.sync.dma_start(out=id_sb[:], in_=ident[:])
            nc.sync.dma_start(out=iota_sb[:], in_=iotaP[:])
            nc.sync.dma_start(out=io16_sb[:], in_=io16[:])

            wts_sb = cpool.tile([P, N_TILES * N_COMPRESS], f32)

            def tok(t):
                return slice(t * P, (t + 1) * P)

            # ============ A: router softmax (all tiles) ============
            for t in range(N_TILES):
                rps_full = psw.tile([P, 2048], f32, space="PSUM", tag="w", name=f"rps_{t}")
                rps = rps_full[:, :N_COMPRESS]
                for dc in range(N_DC):
                    nc.tensor.matmul(
                        out=rps,
                        lhsT=xT_sb[:, dc * TOK_PER_CORE + t * P:dc * TOK_PER_CORE + (t + 1) * P],
                        rhs=rw_sb[:, dc * N_COMPRESS:(dc + 1) * N_COMPRESS],
                        start=(dc == 0), stop=(dc == N_DC - 1))
                w_ = wts_sb[:, t * N_COMPRESS:(t + 1) * N_COMPRESS]
                mx = spool.tile([P, 1], f32, tag="mx")
                sm = spool.tile([P, 1], f32, tag="sm")
                ex = spool.tile([P, N_COMPRESS], f32, tag="ex")
                nc.vector.tensor_reduce(out=mx[:], in_=rps, op=AL.max, axis=mybir.AxisListType.X)
                nc.vector.tensor_scalar(out=ex[:], in0=rps, scalar1=mx[:, :1], scalar2=None,
                                        op0=AL.subtract)
                nc.scalar.activation(out=ex[:], in_=ex[:], func=mybir.ActivationFunctionType.Exp,
                                     accum_out=sm[:, :1])
                rcp = spool.tile([P, 1], f32, tag="rcp")
                nc.vector.reciprocal(out=rcp[:], in_=sm[:, :1])
                nc.vector.tensor_scalar(out=w_, in0=ex[:], scalar1=rcp[:, :1], scalar2=None,
                                        op0=AL.mult)

            # ============ B: y (f32, Wg streamed) + Q combine on DVE ============
            Q_sb = cpool.tile([P, N_TILES * P], f32)        # exact Q per tile
            for gh in range(2):
                ypsl = [psw.tile([P, 2048], f32, space="PSUM", tag="w", name=f"yps_{gh}_{s}")
                        for s in range(2)]                  # slot s holds tiles 2s,2s+1
                for dc in range(N_DC):
                    wgb = wgpool.tile([P, 1024], f32, tag="wgb", name=f"wg_{gh}_{dc}")
                    nc.sync.dma_start(
                        out=wgb[:], in_=Wg[:, (gh * N_DC + dc) * 1024:(gh * N_DC + dc + 1) * 1024])
                    for t in range(N_TILES):
                        for g2_ in range(2):
                            nc.tensor.matmul(
                                out=ypsl[t // 2][:, (t % 2) * 1024 + g2_ * 512:
                                                 (t % 2) * 1024 + (g2_ + 1) * 512],
                                lhsT=xT_sb[:, dc * TOK_PER_CORE + t * P:
                                           dc * TOK_PER_CORE + (t + 1) * P],
                                rhs=wgb[:, g2_ * 512:(g2_ + 1) * 512],
                                start=(dc == 0), stop=(dc == N_DC - 1))
                # combine: Q[t] += sum_n w_n * y_n  (8 neurons in this half)
                for t in range(N_TILES):
                    q_ = Q_sb[:, t * P:(t + 1) * P]
                    for ln in range(8):
                        n = gh * 8 + ln
                        wcol = wts_sb[:, t * N_COMPRESS + n:t * N_COMPRESS + n + 1]
                        ypart = ypsl[t // 2][:, (t % 2) * 1024 + ln * P:(t % 2) * 1024 + (ln + 1) * P]
                        if gh == 0 and ln == 0:
                            nc.vector.tensor_scalar(out=q_, in0=ypart, scalar1=wcol,
                                                    scalar2=None, op0=AL.mult)
                        else:
                            nc.vector.scalar_tensor_tensor(out=q_, in0=ypart, scalar=wcol,
                                                           in1=q_, op0=AL.mult, op1=AL.add)

            def emit_tile(t):
                w_ = wts_sb[:, t * N_COMPRESS:(t + 1) * N_COMPRESS]
                # qt16 = f16(scale * Q^T) via PE transpose; qf32 = scale * Q
                qtps_full = psw.tile([P, 2048], f32, space="PSUM", tag="w", name=f"qtps_{t}")
                qtps = qtps_full[:, :P]
                nc.tensor.transpose(out=qtps, in_=Q_sb[:, tok(t)], identity=idf_sb[:])
                qt16 = qpool.tile([P, P], f16, tag="qt16", name=f"qt16_{t}")
                nc.scalar.activation(out=qt16[:], in_=qtps,
                                     func=mybir.ActivationFunctionType.Copy,
                                     scale=float(SCALE * SSEL))
                if dbg:
                    nc.sync.dma_start(out=d_qt[:, tok(t)], in_=qt16[:])
                qf32 = qpool.tile([P, P], f32, tag="qf32", name=f"qf32_{t}")
                nc.scalar.activation(out=qf32[:], in_=Q_sb[:, tok(t)],
                                     func=mybir.ActivationFunctionType.Copy,
                                     scale=float(SCALE * SSEL))

                # ============ C: scores + L1 pair-reduce (per half-tile) ============
                bm = sp1pool.tile([P, N_BLK], f16, tag="bm", name=f"bm_{t}")
                NBH = N_BLK // 2                         # 128 blocks per half
                for h in range(2):
                    sp1 = sp1pool.tile([P, N_PAIR // 2], f16, tag="sp1", name=f"sp1_{t}_{h}")
                    for wl in range(N_WIN // 2):
                        w = h * (N_WIN // 2) + wl
                        wps = psw.tile([P, WIN], f32, space="PSUM", tag="w", name=f"wps_{t}_{w}")
                        for j in range(4):
                            nc.tensor.matmul(
                                out=wps[:, j * 512:(j + 1) * 512],
                                lhsT=qt16[:],
                                rhs=kt_sb[:, w * WIN + j * 512:w * WIN + (j + 1) * 512],
                                start=True, stop=True)
                        sp1w = sp1[:, wl * HALF:(wl + 1) * HALF]
                        if wl < Z_A // 2:
                            stg = stgpool.tile([P, WIN], f16, tag="stg", name=f"stgA_{t}_{w}")
                            nc.scalar.copy(out=stg[:], in_=wps[:])
                            nc.vector.tensor_tensor(out=sp1w, in0=stg[:, :HALF],
                                                    in1=stg[:, HALF:], op=AL.max)
                        else:
                            stg = stgpool.tile([P, WIN], f16, tag="stg", name=f"stgB_{t}_{w}")
                            nc.scalar.copy(out=stg[:, :HALF], in_=wps[:, HALF:])
                            nc.vector.tensor_tensor(out=sp1w, in0=wps[:, :HALF],
                                                    in1=stg[:, :HALF], op=AL.max)

                    # spill this half: DRAM rows p*256 + h*128 + local
                    o_sp = _copy.copy(sp1D[t][:])
                    o_sp.offset = h * NBH * BLK
                    o_sp.ap = mybir.VecI64Pair([[N_BLK * BLK, P], [1, NBH * BLK]])
                    nc.sync.dma_start(out=o_sp, in_=sp1[:])

                    # tree: blockmax for this half's 128 blocks
                    tw = sp1
                    for wd in (32, 16, 8, 4, 2):
                        tag = "trA" if wd in (32, 8, 2) else "trB"
                        nxt = sp1pool.tile([P, NBH * wd], f16, tag=tag, name=f"tr{wd}_{t}_{h}")
                        s3 = tw[:].rearrange("p (b w) -> p b w", b=NBH)
                        nc.vector.tensor_tensor(out=nxt[:].rearrange("p (b w) -> p b w", b=NBH),
                                                in0=s3[:, :, 0:wd], in1=s3[:, :, wd:2 * wd],
                                                op=AL.max)
                        tw = nxt
                    s3 = tw[:].rearrange("p (b w) -> p b w", b=NBH)
                    nc.vector.tensor_tensor(
                        out=bm[:, h * NBH:(h + 1) * NBH].rearrange("p (b w) -> p b w", b=NBH),
                        in0=s3[:, :, 0:1], in1=s3[:, :, 1:2], op=AL.max)
                if dbg:
                    nc.sync.dma_start(out=d_bm[:, t * N_BLK:(t + 1) * N_BLK], in_=bm[:])

                # ============ L2: top-9 blocks (8 + tie slack) ============
                bv8 = spool.tile([P, 8], f16, tag="bv8")
                bu8 = spool.tile([P, 8], u32, tag="bu8")
                nc.vector.max(out=bv8[:], in_=bm[:])
                nc.vector.max_index(out=bu8[:], in_max=bv8[:], in_values=bm[:])
                bmr = spool.tile([P, N_BLK], f16, tag="bmr")
                nc.vector.match_replace(out=bmr[:], in_to_replace=bv8[:], in_values=bm[:],
                                        imm_value=-60000.0)
                bv9 = spool.tile([P, 8], f16, tag="bv9")
                bu9 = spool.tile([P, 8], u32, tag="bu9")
                nc.vector.max(out=bv9[:], in_=bmr[:])
                nc.vector.max_index(out=bu9[:], in_max=bv9[:], in_values=bmr[:])
                bif = spool.tile([P, NBSEL], f32, tag="bif")
                nc.vector.tensor_copy(out=bif[:, :8], in_=bu8[:])
                nc.vector.tensor_copy(out=bif[:, 8:NBSEL], in_=bu9[:, :NBSEL - 8])
                rowb = spool.tile([P, 1], f32, tag="rowb")
                nc.vector.tensor_scalar(out=rowb[:], in0=iota_sb[:], scalar1=float(N_BLK),
                                        scalar2=None, op0=AL.mult)
                gidx = spool.tile([P, NBSEL], f32, tag="gidx")
                nc.vector.tensor_scalar(out=gidx[:], in0=bif[:], scalar1=rowb[:, :1],
                                        scalar2=None, op0=AL.add)
                gidx_u = spool.tile([P, NBSEL], u32, tag="gidx_u")
                nc.vector.tensor_copy(out=gidx_u[:], in_=gidx[:])

                # ============ G1: gather winning blocks ============
                g1 = g1pool.tile([P, NBSEL * BLK], f16, tag="g1", name=f"g1_{t}")
                for s in range(NBSEL):
                    nc.gpsimd.indirect_dma_start(
                        out=g1[:, s * BLK:(s + 1) * BLK], out_offset=None,
                        in_=sp1D[t][:],
                        in_offset=bass.IndirectOffsetOnAxis(ap=gidx_u[:, s:s + 1], axis=0))
                if dbg:
                    nc.sync.dma_start(out=d_g1[:, t * 512:(t + 1) * 512], in_=g1[:, :512])

                # ============ L3: top-10 pairs (8 + tie slack) ============
                pv8 = spool.tile([P, 8], f16, tag="pv8")
                pp8 = spool.tile([P, 8], u32, tag="pp8")
                nc.vector.max(out=pv8[:], in_=g1[:])
                nc.vector.max_index(out=pp8[:], in_max=pv8[:], in_values=g1[:])
                g1r = g1pool.tile([P, NBSEL * BLK], f16, tag="g1r", name=f"g1r_{t}")
                nc.vector.match_replace(out=g1r[:], in_to_replace=pv8[:], in_values=g1[:],
                                        imm_value=-60000.0)
                pv9 = spool.tile([P, 8], f16, tag="pv9")
                pp9 = spool.tile([P, 8], u32, tag="pp9")
                nc.vector.max(out=pv9[:], in_=g1r[:])
                nc.vector.max_index(out=pp9[:], in_max=pv9[:], in_values=g1r[:])
                ppN = spool.tile([P, NPSEL], u32, tag="ppN")
                nc.vector.tensor_copy(out=ppN[:, :8], in_=pp8[:])
                nc.vector.tensor_copy(out=ppN[:, 8:NPSEL], in_=pp9[:, :NPSEL - 8])
                slot_u = spool.tile([P, NPSEL], u32, tag="slot_u")
                nc.vector.tensor_scalar(out=slot_u[:], in0=ppN[:], scalar1=6, scalar2=None,
                                        op0=AL.logical_shift_right)
                slotf = spool.tile([P, NPSEL], f32, tag="slotf")
                ppNf = spool.tile([P, NPSEL], f32, tag="ppNf")
                nc.vector.tensor_copy(out=slotf[:], in_=slot_u[:])
                nc.vector.tensor_copy(out=ppNf[:], in_=ppN[:])
                # off = pp - 64*slot
                offN = spool.tile([P, NPSEL], f32, tag="offN")
                nc.vector.scalar_tensor_tensor(out=offN[:], in0=slotf[:], scalar=-float(BLK),
                                               in1=ppNf[:], op0=AL.mult, op1=AL.add)
                # blk by slot: is_equal-accum over NBSEL slots
                blkj = spool.tile([P, NPSEL], f32, tag="blkj")
                junk = spool.tile([P, NBSEL], f32, tag="junk9")
                for j in range(NPSEL):
                    nc.vector.scalar_tensor_tensor(
                        out=junk[:], in0=io16_sb[:, :NBSEL], scalar=slotf[:, j:j + 1],
                        in1=bif[:], op0=AL.is_equal, op1=AL.mult,
                        accum_out=blkj[:, j:j + 1])
                # pairidx = blk*64 + off
                pj = spool.tile([P, NPSEL], f32, tag="pj")
                nc.vector.scalar_tensor_tensor(out=pj[:], in0=blkj[:], scalar=float(BLK),
                                               in1=offN[:], op0=AL.mult, op1=AL.add)
                if dbg:
                    nc.sync.dma_start(out=d_pj[:, t * NPSEL:(t + 1) * NPSEL], in_=pj[:])
                pj_u = spool.tile([P, NPSEL], u32, tag="pj_u")
                nc.vector.tensor_copy(out=pj_u[:], in_=pj[:])

                # ============ G2 + exact rescore (f32) ============
                g2 = g2pool.tile([P, NPSEL * 2 * RANK], f32, tag="g2", name=f"g2_{t}")
                for s in range(NPSEL):
                    nc.gpsimd.indirect_dma_start(
                        out=g2[:, s * 256:(s + 1) * 256], out_offset=None,
                        in_=KP32[:],
                        in_offset=bass.IndirectOffsetOnAxis(ap=pj_u[:, s:s + 1], axis=0))
                rsc = spool.tile([P, NCAND], f32, tag="rsc")
                HC = NCAND // 2
                for hc in range(2):
                    prod = prpool.tile([P, HC * RANK], f32, tag="prod", name=f"prod_{t}_{hc}")
                    qb = _copy.copy(qf32[:])
                    qb.ap = mybir.VecI64Pair([[qb.ap[0][0], P], [0, HC], [1, RANK]])
                    nc.vector.tensor_tensor(
                        out=prod[:].rearrange("p (c r) -> p c r", c=HC),
                        in0=g2[:, hc * HC * RANK:(hc + 1) * HC * RANK]
                            .rearrange("p (c r) -> p c r", c=HC),
                        in1=qb, op=AL.mult)
                    nc.vector.tensor_reduce(out=rsc[:, hc * HC:(hc + 1) * HC],
                                            in_=prod[:].rearrange("p (c r) -> p c r", c=HC),
                                            op=AL.add, axis=mybir.AxisListType.X)
                if dbg:
                    nc.sync.dma_start(out=d_rs[:, t * NCAND:(t + 1) * NCAND], in_=rsc[:])

                # ============ exact top-8 of NCAND (f32) + keys ============
                v8 = spool.tile([P, 8], f32, tag="v8")
                s8 = spool.tile([P, 8], u32, tag="s8")
                nc.vector.max(out=v8[:], in_=rsc[:])
                nc.vector.max_index(out=s8[:], in_max=v8[:], in_values=rsc[:])
                j8u = spool.tile([P, 8], u32, tag="j8u")
                nc.vector.tensor_scalar(out=j8u[:], in0=s8[:], scalar1=1, scalar2=None,
                                        op0=AL.logical_shift_right)
                j8f = spool.tile([P, 8], f32, tag="j8f")
                s8f = spool.tile([P, 8], f32, tag="s8f")
                nc.vector.tensor_copy(out=j8f[:], in_=j8u[:])
                nc.vector.tensor_copy(out=s8f[:], in_=s8[:])
                m8 = spool.tile([P, 8], f32, tag="m8")      # member = s - 2*j
                nc.vector.scalar_tensor_tensor(out=m8[:], in0=j8f[:], scalar=-2.0,
                                               in1=s8f[:], op0=AL.mult, op1=AL.add)
                # pairidx by j
                psel = spool.tile([P, 8], f32, tag="psel")
                junk2 = spool.tile([P, NPSEL], f32, tag="junk10")
                for j in range(8):
                    nc.vector.scalar_tensor_tensor(
                        out=junk2[:], in0=io16_sb[:, :NPSEL], scalar=j8f[:, j:j + 1], in1=pj[:],
                        op0=AL.is_equal, op1=AL.mult, accum_out=psel[:, j:j + 1])
                psel_u = spool.tile([P, 8], u32, tag="psel_u")
                wsel_u = spool.tile([P, 8], u32, tag="wsel_u")
                nc.vector.tensor_copy(out=psel_u[:], in_=psel[:])
                nc.vector.tensor_scalar(out=wsel_u[:], in0=psel_u[:], scalar1=10, scalar2=None,
                                        op0=AL.logical_shift_right)
                wself = spool.tile([P, 8], f32, tag="wself")
                nc.vector.tensor_copy(out=wself[:], in_=wsel_u[:])
                # key = psel + 1024*wsel + 1024*m
                keyf = spool.tile([P, 8], f32, tag="keyf")
                nc.vector.scalar_tensor_tensor(out=keyf[:], in0=wself[:], scalar=1024.0,
                                               in1=psel[:], op0=AL.mult, op1=AL.add)
                nc.vector.scalar_tensor_tensor(out=keyf[:], in0=m8[:], scalar=1024.0,
                                               in1=keyf[:], op0=AL.mult, op1=AL.add)
                if dbg:
                    nc.sync.dma_start(out=d_key[:, t * 8:(t + 1) * 8], in_=keyf[:])
                key_u = spool.tile([P, 8], u32, tag="key_u")
                nc.vector.tensor_copy(out=key_u[:], in_=keyf[:])

                # softmax over v8 (descending, v8[0] is max); exp scale 1/SSEL
                w8 = spool.tile([P, 8], f32, tag="w8")
                sm8 = spool.tile([P, 1], f32, tag="sm8")
                nc.vector.tensor_scalar(out=w8[:], in0=v8[:], scalar1=v8[:, :1], scalar2=None,
                                        op0=AL.subtract)
                nc.scalar.activation(out=w8[:], in_=w8[:], func=mybir.ActivationFunctionType.Exp,
                                     scale=float(1.0 / SSEL), accum_out=sm8[:, :1])
                rcp8 = spool.tile([P, 1], f32, tag="rcp8")
                nc.vector.reciprocal(out=rcp8[:], in_=sm8[:, :1])
                nc.vector.tensor_scalar(out=w8[:], in0=w8[:], scalar1=rcp8[:, :1], scalar2=None,
                                        op0=AL.mult)
                if dbg:
                    nc.sync.dma_start(out=d_w8[:, t * 8:(t + 1) * 8], in_=w8[:])

                # ============ G3 + Vsum via diag matmuls ============
                g3 = g3pool.tile([P, 8 * D_MODEL], f16, tag="g3", name=f"g3_{t}")
                for s in range(8):
                    nc.gpsimd.indirect_dma_start(
                        out=g3[:, s * D_MODEL:(s + 1) * D_MODEL], out_offset=None,
                        in_=VD16[:],
                        in_offset=bass.IndirectOffsetOnAxis(ap=key_u[:, s:s + 1], axis=0))
                dg8 = dgpool.tile([P, 8 * P], f16, tag="dg8", name=f"dg8_{t}")
                for s in range(8):
                    nc.vector.tensor_scalar(out=dg8[:, s * P:(s + 1) * P], in0=id_sb[:],
                                            scalar1=w8[:, s:s + 1], scalar2=None, op0=AL.mult)
                accps_full = psw.tile([P, 2048], f32, space="PSUM", tag="w", name=f"accps_{t}")
                accps = accps_full[:, :D_MODEL]
                for h in range(2):
                    for s in range(8):
                        nc.tensor.matmul(
                            out=accps[:, h * 512:(h + 1) * 512],
                            lhsT=dg8[:, s * P:(s + 1) * P],
                            rhs=g3[:, s * D_MODEL + h * 512:s * D_MODEL + (h + 1) * 512],
                            start=(s == 0), stop=(s == 7))
                accf = apool.tile([P, D_MODEL], f32, tag="accf", name=f"accf_{t}")
                nc.scalar.copy(out=accf[:], in_=accps)
                nc.sync.dma_start(out=out[t * P:(t + 1) * P, :], in_=accf[:])
                if dbg:
                    nc.sync.dma_start(out=d_wts[:], in_=wts_sb[:])

            for t in range(N_TILES):
                emit_tile(t)

    nc.compile()
    return nc


_NC_CACHE = {}


def _get_nc(dbg=False):
    if dbg not in _NC_CACHE:
        _NC_CACHE[dbg] = _build(dbg)
    return _NC_CACHE[dbg]


def _prep_in_maps(x, router_w, compress_neurons, knowledge_K, knowledge_V):
    x = np.asarray(x, dtype=np.float32).reshape(B * S, D_MODEL)
    rwT = np.asarray(router_w, dtype=np.float32).T          # [1024, 16]
    rw_r = np.ascontiguousarray(
        rwT.reshape(N_DC, P, N_COMPRESS).transpose(1, 0, 2).reshape(P, N_DC * N_COMPRESS))
    cn = np.asarray(compress_neurons, dtype=np.float32)     # [16, 1024, 128]
    # Wg[p, (gh*8+dc)*1024 + ln*128 + r] = cn[8*gh+ln, dc*128+p, r]
    Wg = np.ascontiguousarray(
        cn.reshape(2, 8, N_DC, P, RANK).transpose(3, 0, 2, 1, 4).reshape(P, 2 * N_DC * 1024))
    K = np.asarray(knowledge_K, dtype=np.float32)
    KT16 = np.ascontiguousarray(K.T).astype(np.float16)     # [128, 32768]
    # KP32[w*1024+j] = (K[2048w+j], K[2048w+1024+j]) in f32
    KP32 = np.ascontiguousarray(
        K.reshape(N_WIN, 2, HALF, RANK).transpose(0, 2, 1, 3).reshape(N_PAIR, 2 * RANK))
    V16 = np.asarray(knowledge_V, dtype=np.float32).astype(np.float16)
    ident = np.eye(P, dtype=np.float16)
    idf32 = np.eye(P, dtype=np.float32)
    iotaP = np.arange(P, dtype=np.float32).reshape(P, 1)
    io16 = np.broadcast_to(np.arange(24, dtype=np.float32), (P, 24)).copy()

    in_maps = []
    for c in range(N_CORES):
        xs = x[c * TOK_PER_CORE:(c + 1) * TOK_PER_CORE]
        xTc = np.ascontiguousarray(
            xs.T.reshape(N_DC, P, TOK_PER_CORE).transpose(1, 0, 2).reshape(P, N_DC * TOK_PER_CORE))
        in_maps.append(dict(xT=xTc, rw=rw_r, Wg=Wg, KT16=KT16, KP32=KP32, VD16=V16,
                            ident=ident, idf32=idf32, iotaP=iotaP, io16=io16))
    return in_maps


def _ensure_ntff_hook():
    import sys as _sys
    import types as _types
    if "antenv.axon_hooks" in _sys.modules:
        return
    try:
        import antenv.axon_hooks  # noqa: F401
        return
    except ImportError:
        pass
    mod = _types.ModuleType("antenv.axon_hooks")
    _state = {"hook": None}
    mod.set_axon_ntff_profile_hook = lambda h: _state.__setitem__("hook", h)
    mod.get_axon_ntff_profile_hook = lambda: _state["hook"]
    _sys.modules["antenv.axon_hooks"] = mod
    try:
        from trn_agent_boot.trn_boot import _ntff_profile_via_ctypes
        mod.set_axon_ntff_profile_hook(_ntff_profile_via_ctypes("/opt/axon/libaxon_pjrt.so"))
    except Exception:
        pass


def _run(inputs, trace=False, dbg=False):
    if trace:
        _ensure_ntff_hook()
    nc = _get_nc(dbg)
    in_maps = _prep_in_maps(**inputs)
    res = run_bass_kernel_spmd(nc, in_maps, core_ids=list(range(N_CORES)), trace=trace)
    out = np.concatenate([res.results[c]["out"] for c in range(N_CORES)], axis=0)
    return out.reshape(B, S, D_MODEL).astype(np.float32), res


def kernel(x, router_w, compress_neurons, knowledge_K, knowledge_V):
    out, _ = _run(dict(x=x, router_w=router_w, compress_neurons=compress_neurons,
                       knowledge_K=knowledge_K, knowledge_V=knowledge_V))
    return out


# revision 28
# speedup vs baseline: 1.2554x; 1.2554x over previous
"""NeuronMemory retrieval kernel v4 for 8 TRN2 NeuronCores.

Data-parallel over tokens (512/core, 4 tiles of 128). Per-core, per tile:
  A:  router scores (PE) + softmax -> wts f32 [128,16]
  B:  y = x @ W_n for 16 neurons (PE, f16, 32 matmuls) -> y16 f16 SBUF
      QT[r,tok] = sum_n diag(w_n)-weighted y via 16 PE matmuls (diag as rhs)
      qt16 = f16(QT * SCALE); q16 = transpose(qt16) via PE
  C:  16 windows of 2048 keys: scores = qt16^T @ KT16 (PE, f16) -> PSUM f32
      L1 pair-reduce: sp1[w*1024+j] = max(s[2048w+j], s[2048w+1024+j])
        plan A (Z_A windows): scalar stages whole window f16, DVE TT 2x
        plan B (rest):        scalar stages 2nd half,     DVE TT(PSUM,SBUF)
  tree: sp1 [128,16384] -> blockmax bm [128,256] (64 pairs/block) on DVE
  spill: sp1 -> DRAM rows [128*256, 64] f16 (one 4MB DMA)
  L2: top-8 blocks/token (max8+fi8 on bm)
  G1: gather 8 winning block-rows (64 pair-values each) -> g1 [128,512]
  L3: top-8 pairs/token (max8+fi8 on g1) -> pairidx[8] (global pair ids)
  G2: gather 8 K-pair rows (256 f16) from KP16 -> g2; rescore 16 cands
      exactly on DVE (TT mult + TR add vs q16) -> rsc f32 -> f16
  top8: max8+fi8 on rsc16 -> 8 winners; member bit + key reconstruction;
      softmax(v8) -> w8 f32
  G3: gather 8 V rows (2KB f16) -> g3; Vsum via 8 diag matmuls on PE -> out

Pair rows: KP16[w*1024+j] = (K[2048w+j], K[2048w+1024+j]), both f16.
key = pairidx + 1024*(pairidx>>10) + 1024*member.
"""
import copy as _copy

import numpy as np

import concourse.bacc as bacc
import concourse.bass as bass
import concourse.mybir as mybir
from concourse.tile import TileContext
from concourse.bass_utils import run_bass_kernel_spmd

P = 128
D_MODEL = 1024
RANK = 128
N_COMPRESS = 16
N_KNOWLEDGE = 32768
K_TOP = 8
B, S = 2, 2048
N_CORES = 8
TOK_PER_CORE = (B * S) // N_CORES      # 512
N_TILES = TOK_PER_CORE // P            # 4
N_DC = D_MODEL // P                    # 8
N_WIN = 16                             # score windows per tile
WIN = N_KNOWLEDGE // N_WIN             # 2048 keys per window
HALF = WIN // 2                        # 1024 pairs per window
N_PAIR = N_KNOWLEDGE // 2              # 16384 pairs per tile
BLK = 64                               # pairs per block
N_BLK = N_PAIR // BLK                  # 256 blocks
NBSEL = 9                              # blocks gathered per token (8 + tie slack)
NPSEL = 10                             # pairs rescored per token (8 + tie slack)
NCAND = 2 * NPSEL                      # candidate keys
Z_A = 16                               # plan-A windows per tile (0..16)
SCALE = 1.0 / np.sqrt(np.float32(RANK))
SSEL = 0.5                             # extra selection scale (folded into qt16/q16)

f32 = mybir.dt.float32
f16 = mybir.dt.float16
u32 = mybir.dt.uint32

AL = mybir.AluOpType


def _build(dbg=False):
    nc = bacc.Bacc("TRN2", target_bir_lowering=False, debug=False, num_devices=N_CORES)

    xT = nc.declare_dram_parameter("xT", [P, N_DC * TOK_PER_CORE], f32, isOutput=False)
    rw = nc.declare_dram_parameter("rw", [P, N_DC * N_COMPRESS], f32, isOutput=False)
    # Wg[p, (gh*8+dc)*1024 + ln*128 + r] = cn[8*gh+ln, dc*128+p, r]; streamed
    Wg = nc.declare_dram_parameter("Wg", [P, 2 * N_DC * 1024], f32, isOutput=False)
    KT16 = nc.declare_dram_parameter("KT16", [P, N_KNOWLEDGE], f16, isOutput=False)
    KP32 = nc.declare_dram_parameter("KP32", [N_PAIR, 2 * RANK], f32, isOutput=False)
    VD16 = nc.declare_dram_parameter("VD16", [N_KNOWLEDGE, D_MODEL], f16, isOutput=False)
    ident = nc.declare_dram_parameter("ident", [P, P], f16, isOutput=False)
    idf32 = nc.declare_dram_parameter("idf32", [P, P], f32, isOutput=False)
    iotaP = nc.declare_dram_parameter("iotaP", [P, 1], f32, isOutput=False)
    io16 = nc.declare_dram_parameter("io16", [P, 24], f32, isOutput=False)
    out = nc.declare_dram_parameter("out", [TOK_PER_CORE, D_MODEL], f32, isOutput=True)
    if dbg:
        d_wts = nc.declare_dram_parameter("d_wts", [P, N_TILES * N_COMPRESS], f32, isOutput=True)
        d_qt = nc.declare_dram_parameter("d_qt", [P, N_TILES * P], f16, isOutput=True)
        d_bm = nc.declare_dram_parameter("d_bm", [P, N_TILES * N_BLK], f16, isOutput=True)
        d_g1 = nc.declare_dram_parameter("d_g1", [P, N_TILES * 512], f16, isOutput=True)
        d_pj = nc.declare_dram_parameter("d_pj", [P, N_TILES * NPSEL], f32, isOutput=True)
        d_rs = nc.declare_dram_parameter("d_rs", [P, N_TILES * NCAND], f32, isOutput=True)
        d_key = nc.declare_dram_parameter("d_key", [P, N_TILES * 8], f32, isOutput=True)
        d_w8 = nc.declare_dram_parameter("d_w8", [P, N_TILES * 8], f32, isOutput=True)

    sp1D = {t: nc.dram_tensor(f"sp1D_{t}", [P * N_BLK, BLK], f16) for t in range(N_TILES)}

    with TileContext(nc) as tc:
        with (
            tc.tile_pool(name="const", bufs=1) as cpool,
            tc.tile_pool(name="sp1p", bufs=1) as sp1pool,
            tc.tile_pool(name="stg", bufs=2) as stgpool,
            tc.tile_pool(name="wgs", bufs=2) as wgpool,
            tc.tile_pool(name="dgp", bufs=2) as dgpool,
            tc.tile_pool(name="qp", bufs=4) as qpool,
            tc.tile_pool(name="g1p", bufs=2) as g1pool,
            tc.tile_pool(name="g2p", bufs=1) as g2pool,
            tc.tile_pool(name="g3p", bufs=2) as g3pool,
            tc.tile_pool(name="pr", bufs=1) as prpool,
            tc.tile_pool(name="acc", bufs=2) as apool,
            tc.tile_pool(name="sm", bufs=2) as spool,
            tc.tile_pool(name="psw", bufs=2, space="PSUM") as psw,
        ):
            # ---------------- persistent loads ----------------
            xT_sb = cpool.tile([P, N_DC * TOK_PER_CORE], f32)
            rw_sb = cpool.tile([P, N_DC * N_COMPRESS], f32)
            kt_sb = cpool.tile([P, N_KNOWLEDGE], f16)
            id_sb = cpool.tile([P, P], f16)
            idf_sb = cpool.tile([P, P], f32)
            iota_sb = cpool.tile([P, 1], f32)
            io16_sb = cpool.tile([P, 24], f32)
            nc.sync.dma_start(out=xT_sb[:], in_=xT[:])
            nc.sync.dma_start(out=rw_sb[:], in_=rw[:])
            nc.sync.dma_start(out=idf_sb[:], in_=idf32[:])
            for q in range(4):
                nc.sync.dma_start(out=kt_sb[:, q * 8192:(q + 1) * 8192],
                                  in_=KT16[:, q * 8192:(q + 1) * 8192])
            nc.sync.dma_start(out=id_sb[:], in_=ident[:])
            nc.sync.dma_start(out=iota_sb[:], in_=iotaP[:])
            nc.sync.dma_start(out=io16_sb[:], in_=io16[:])

            wts_sb = cpool.tile([P, N_TILES * N_COMPRESS], f32)

            def tok(t):
                return slice(t * P, (t + 1) * P)

            # ============ A: router softmax (all tiles) ============
            for t in range(N_TILES):
                rps_full = psw.tile([P, 2048], f32, space="PSUM", tag="w", name=f"rps_{t}")
                rps = rps_full[:, :N_COMPRESS]
                for dc in range(N_DC):
                    nc.tensor.matmul(
                        out=rps,
                        lhsT=xT_sb[:, dc * TOK_PER_CORE + t * P:dc * TOK_PER_CORE + (t + 1) * P],
                        rhs=rw_sb[:, dc * N_COMPRESS:(dc + 1) * N_COMPRESS],
                        start=(dc == 0), stop=(dc == N_DC - 1))
                w_ = wts_sb[:, t * N_COMPRESS:(t + 1) * N_COMPRESS]
                mx = spool.tile([P, 1], f32, tag="mx")
                sm = spool.tile([P, 1], f32, tag="sm")
                ex = spool.tile([P, N_COMPRESS], f32, tag="ex")
                nc.vector.tensor_reduce(out=mx[:], in_=rps, op=AL.max, axis=mybir.AxisListType.X)
                nc.vector.tensor_scalar(out=ex[:], in0=rps, scalar1=mx[:, :1], scalar2=None,
                                        op0=AL.subtract)
                nc.scalar.activation(out=ex[:], in_=ex[:], func=mybir.ActivationFunctionType.Exp,
                                     accum_out=sm[:, :1])
                rcp = spool.tile([P, 1], f32, tag="rcp")
                nc.vector.reciprocal(out=rcp[:], in_=sm[:, :1])
                nc.vector.tensor_scalar(out=w_, in0=ex[:], scalar1=rcp[:, :1], scalar2=None,
                                        op0=AL.mult)

            # ============ B: y (f32, Wg streamed) + Q combine on DVE ============
            Q_sb = cpool.tile([P, N_TILES * P], f32)        # exact Q per tile
            for gh in range(2):
                ypsl = [psw.tile([P, 2048], f32, space="PSUM", tag="w", name=f"yps_{gh}_{s}")
                        for s in range(2)]                  # slot s holds tiles 2s,2s+1
                for dc in range(N_DC):
                    wgb = wgpool.tile([P, 1024], f32, tag="wgb", name=f"wg_{gh}_{dc}")
                    nc.sync.dma_start(
                        out=wgb[:], in_=Wg[:, (gh * N_DC + dc) * 1024:(gh * N_DC + dc + 1) * 1024])
                    for t in range(N_TILES):
                        for g2_ in range(2):
                            nc.tensor.matmul(
                                out=ypsl[t // 2][:, (t % 2) * 1024 + g2_ * 512:
                                                 (t % 2) * 1024 + (g2_ + 1) * 512],
                                lhsT=xT_sb[:, dc * TOK_PER_CORE + t * P:
                                           dc * TOK_PER_CORE + (t + 1) * P],
                                rhs=wgb[:, g2_ * 512:(g2_ + 1) * 512],
                                start=(dc == 0), stop=(dc == N_DC - 1))
                # combine: Q[t] += sum_n w_n * y_n  (8 neurons in this half)
                for t in range(N_TILES):
                    q_ = Q_sb[:, t * P:(t + 1) * P]
                    for ln in range(8):
                        n = gh * 8 + ln
                        wcol = wts_sb[:, t * N_COMPRESS + n:t * N_COMPRESS + n + 1]
                        ypart = ypsl[t // 2][:, (t % 2) * 1024 + ln * P:(t % 2) * 1024 + (ln + 1) * P]
                        if gh == 0 and ln == 0:
                            nc.vector.tensor_scalar(out=q_, in0=ypart, scalar1=wcol,
                                                    scalar2=None, op0=AL.mult)
                        else:
                            nc.vector.scalar_tensor_tensor(out=q_, in0=ypart, scalar=wcol,
                                                           in1=q_, op0=AL.mult, op1=AL.add)

            # ============ Q: qt16/qf32 for all tiles (PE transpose) ============
            TS = {}
            for t in range(N_TILES):
                qtps_full = psw.tile([P, 2048], f32, space="PSUM", tag="w", name=f"qtps_{t}")
                qtps = qtps_full[:, :P]
                nc.tensor.transpose(out=qtps, in_=Q_sb[:, tok(t)], identity=idf_sb[:])
                qt16 = qpool.tile([P, P], f16, tag="qt16", name=f"qt16_{t}")
                nc.scalar.activation(out=qt16[:], in_=qtps,
                                     func=mybir.ActivationFunctionType.Copy,
                                     scale=float(SCALE * SSEL))
                if dbg:
                    nc.sync.dma_start(out=d_qt[:, tok(t)], in_=qt16[:])
                qf32 = qpool.tile([P, P], f32, tag="qf32", name=f"qf32_{t}")
                nc.scalar.activation(out=qf32[:], in_=Q_sb[:, tok(t)],
                                     func=mybir.ActivationFunctionType.Copy,
                                     scale=float(SCALE * SSEL))
                TS[t] = dict(qt16=qt16, qf32=qf32)

            def emit_C(t):
                qt16 = TS[t]["qt16"]
                # ============ C: scores + L1 pair-reduce (per half-tile) ============
                bm = dgpool.tile([P, N_BLK], f16, tag="bm", name=f"bm_{t}")
                TS[t]["bm"] = bm
                NBH = N_BLK // 2                         # 128 blocks per half
                for h in range(2):
                    sp1 = sp1pool.tile([P, N_PAIR // 2], f16, tag="sp1", name=f"sp1_{t}_{h}")
                    for wl in range(N_WIN // 2):
                        w = h * (N_WIN // 2) + wl
                        wps = psw.tile([P, WIN], f32, space="PSUM", tag="w", name=f"wps_{t}_{w}")
                        for j in range(4):
                            nc.tensor.matmul(
                                out=wps[:, j * 512:(j + 1) * 512],
                                lhsT=qt16[:],
                                rhs=kt_sb[:, w * WIN + j * 512:w * WIN + (j + 1) * 512],
                                start=True, stop=True)
                        sp1w = sp1[:, wl * HALF:(wl + 1) * HALF]
                        if wl < Z_A // 2:
                            stg = stgpool.tile([P, WIN], f16, tag="stg", name=f"stgA_{t}_{w}")
                            nc.scalar.copy(out=stg[:], in_=wps[:])
                            nc.vector.tensor_tensor(out=sp1w, in0=stg[:, :HALF],
                                                    in1=stg[:, HALF:], op=AL.max)
                        else:
                            stg = stgpool.tile([P, WIN], f16, tag="stg", name=f"stgB_{t}_{w}")
                            nc.scalar.copy(out=stg[:, :HALF], in_=wps[:, HALF:])
                            nc.vector.tensor_tensor(out=sp1w, in0=wps[:, :HALF],
                                                    in1=stg[:, :HALF], op=AL.max)

                    # spill this half: DRAM rows p*256 + h*128 + local
                    o_sp = _copy.copy(sp1D[t][:])
                    o_sp.offset = h * NBH * BLK
                    o_sp.ap = mybir.VecI64Pair([[N_BLK * BLK, P], [1, NBH * BLK]])
                    nc.sync.dma_start(out=o_sp, in_=sp1[:])

                    # tree: blockmax for this half's 128 blocks
                    tw = sp1
                    for wd in (32, 16, 8, 4, 2):
                        tag = "trA" if wd in (32, 8, 2) else "trB"
                        nxt = sp1pool.tile([P, NBH * wd], f16, tag=tag, name=f"tr{wd}_{t}_{h}")
                        s3 = tw[:].rearrange("p (b w) -> p b w", b=NBH)
                        nc.vector.tensor_tensor(out=nxt[:].rearrange("p (b w) -> p b w", b=NBH),
                                                in0=s3[:, :, 0:wd], in1=s3[:, :, wd:2 * wd],
                                                op=AL.max)
                        tw = nxt
                    s3 = tw[:].rearrange("p (b w) -> p b w", b=NBH)
                    nc.vector.tensor_tensor(
                        out=bm[:, h * NBH:(h + 1) * NBH].rearrange("p (b w) -> p b w", b=NBH),
                        in0=s3[:, :, 0:1], in1=s3[:, :, 1:2], op=AL.max)
                if dbg:
                    nc.sync.dma_start(out=d_bm[:, t * N_BLK:(t + 1) * N_BLK], in_=bm[:])

            def emit_tail(t):
                bm = TS[t]["bm"]
                qf32 = TS[t]["qf32"]
                # ============ L2: top-9 blocks (8 + tie slack) ============
                bv8 = spool.tile([P, 8], f16, tag="bv8")
                bu8 = spool.tile([P, 8], u32, tag="bu8")
                nc.vector.max(out=bv8[:], in_=bm[:])
                nc.vector.max_index(out=bu8[:], in_max=bv8[:], in_values=bm[:])
                bmr = spool.tile([P, N_BLK], f16, tag="bmr")
                nc.vector.match_replace(out=bmr[:], in_to_replace=bv8[:], in_values=bm[:],
                                        imm_value=-60000.0)
                bv9 = spool.tile([P, 8], f16, tag="bv9")
                bu9 = spool.tile([P, 8], u32, tag="bu9")
                nc.vector.max(out=bv9[:], in_=bmr[:])
                nc.vector.max_index(out=bu9[:], in_max=bv9[:], in_values=bmr[:])
                bif = spool.tile([P, NBSEL], f32, tag="bif")
                nc.vector.tensor_copy(out=bif[:, :8], in_=bu8[:])
                nc.vector.tensor_copy(out=bif[:, 8:NBSEL], in_=bu9[:, :NBSEL - 8])
                rowb = spool.tile([P, 1], f32, tag="rowb")
                nc.vector.tensor_scalar(out=rowb[:], in0=iota_sb[:], scalar1=float(N_BLK),
                                        scalar2=None, op0=AL.mult)
                gidx = spool.tile([P, NBSEL], f32, tag="gidx")
                nc.vector.tensor_scalar(out=gidx[:], in0=bif[:], scalar1=rowb[:, :1],
                                        scalar2=None, op0=AL.add)
                gidx_u = spool.tile([P, NBSEL], u32, tag="gidx_u")
                nc.vector.tensor_copy(out=gidx_u[:], in_=gidx[:])

                # ============ G1: gather winning blocks ============
                g1 = g1pool.tile([P, NBSEL * BLK], f16, tag="g1", name=f"g1_{t}")
                for s in range(NBSEL):
                    nc.gpsimd.indirect_dma_start(
                        out=g1[:, s * BLK:(s + 1) * BLK], out_offset=None,
                        in_=sp1D[t][:],
                        in_offset=bass.IndirectOffsetOnAxis(ap=gidx_u[:, s:s + 1], axis=0))
                if dbg:
                    nc.sync.dma_start(out=d_g1[:, t * 512:(t + 1) * 512], in_=g1[:, :512])

                # ============ L3: top-10 pairs (8 + tie slack) ============
                pv8 = spool.tile([P, 8], f16, tag="pv8")
                pp8 = spool.tile([P, 8], u32, tag="pp8")
                nc.vector.max(out=pv8[:], in_=g1[:])
                nc.vector.max_index(out=pp8[:], in_max=pv8[:], in_values=g1[:])
                g1r = g1pool.tile([P, NBSEL * BLK], f16, tag="g1r", name=f"g1r_{t}")
                nc.vector.match_replace(out=g1r[:], in_to_replace=pv8[:], in_values=g1[:],
                                        imm_value=-60000.0)
                pv9 = spool.tile([P, 8], f16, tag="pv9")
                pp9 = spool.tile([P, 8], u32, tag="pp9")
                nc.vector.max(out=pv9[:], in_=g1r[:])
                nc.vector.max_index(out=pp9[:], in_max=pv9[:], in_values=g1r[:])
                ppN = spool.tile([P, NPSEL], u32, tag="ppN")
                nc.vector.tensor_copy(out=ppN[:, :8], in_=pp8[:])
                nc.vector.tensor_copy(out=ppN[:, 8:NPSEL], in_=pp9[:, :NPSEL - 8])
                slot_u = spool.tile([P, NPSEL], u32, tag="slot_u")
                nc.vector.tensor_scalar(out=slot_u[:], in0=ppN[:], scalar1=6, scalar2=None,
                                        op0=AL.logical_shift_right)
                slotf = spool.tile([P, NPSEL], f32, tag="slotf")
                ppNf = spool.tile([P, NPSEL], f32, tag="ppNf")
                nc.vector.tensor_copy(out=slotf[:], in_=slot_u[:])
                nc.vector.tensor_copy(out=ppNf[:], in_=ppN[:])
                # off = pp - 64*slot
                offN = spool.tile([P, NPSEL], f32, tag="offN")
                nc.vector.scalar_tensor_tensor(out=offN[:], in0=slotf[:], scalar=-float(BLK),
                                               in1=ppNf[:], op0=AL.mult, op1=AL.add)
                # blk by slot: is_equal-accum over NBSEL slots
                blkj = spool.tile([P, NPSEL], f32, tag="blkj")
                junk = spool.tile([P, NBSEL], f32, tag="junk9")
                for j in range(NPSEL):
                    nc.vector.scalar_tensor_tensor(
                        out=junk[:], in0=io16_sb[:, :NBSEL], scalar=slotf[:, j:j + 1],
                        in1=bif[:], op0=AL.is_equal, op1=AL.mult,
                        accum_out=blkj[:, j:j + 1])
                # pairidx = blk*64 + off
                pj = spool.tile([P, NPSEL], f32, tag="pj")
                nc.vector.scalar_tensor_tensor(out=pj[:], in0=blkj[:], scalar=float(BLK),
                                               in1=offN[:], op0=AL.mult, op1=AL.add)
                if dbg:
                    nc.sync.dma_start(out=d_pj[:, t * NPSEL:(t + 1) * NPSEL], in_=pj[:])
                pj_u = spool.tile([P, NPSEL], u32, tag="pj_u")
                nc.vector.tensor_copy(out=pj_u[:], in_=pj[:])

                # ============ G2 + exact rescore (f32) ============
                g2 = g2pool.tile([P, NPSEL * 2 * RANK], f32, tag="g2", name=f"g2_{t}")
                for s in range(NPSEL):
                    nc.gpsimd.indirect_dma_start(
                        out=g2[:, s * 256:(s + 1) * 256], out_offset=None,
                        in_=KP32[:],
                        in_offset=bass.IndirectOffsetOnAxis(ap=pj_u[:, s:s + 1], axis=0))
                rsc = spool.tile([P, NCAND], f32, tag="rsc")
                HC = NCAND // 2
                for hc in range(2):
                    prod = prpool.tile([P, HC * RANK], f32, tag="prod", name=f"prod_{t}_{hc}")
                    qb = _copy.copy(qf32[:])
                    qb.ap = mybir.VecI64Pair([[qb.ap[0][0], P], [0, HC], [1, RANK]])
                    nc.vector.tensor_tensor(
                        out=prod[:].rearrange("p (c r) -> p c r", c=HC),
                        in0=g2[:, hc * HC * RANK:(hc + 1) * HC * RANK]
                            .rearrange("p (c r) -> p c r", c=HC),
                        in1=qb, op=AL.mult)
                    nc.vector.tensor_reduce(out=rsc[:, hc * HC:(hc + 1) * HC],
                                            in_=prod[:].rearrange("p (c r) -> p c r", c=HC),
                                            op=AL.add, axis=mybir.AxisListType.X)
                if dbg:
                    nc.sync.dma_start(out=d_rs[:, t * NCAND:(t + 1) * NCAND], in_=rsc[:])

                # ============ exact top-8 of NCAND (f32) + keys ============
                v8 = spool.tile([P, 8], f32, tag="v8")
                s8 = spool.tile([P, 8], u32, tag="s8")
                nc.vector.max(out=v8[:], in_=rsc[:])
                nc.vector.max_index(out=s8[:], in_max=v8[:], in_values=rsc[:])
                j8u = spool.tile([P, 8], u32, tag="j8u")
                nc.vector.tensor_scalar(out=j8u[:], in0=s8[:], scalar1=1, scalar2=None,
                                        op0=AL.logical_shift_right)
                j8f = spool.tile([P, 8], f32, tag="j8f")
                s8f = spool.tile([P, 8], f32, tag="s8f")
                nc.vector.tensor_copy(out=j8f[:], in_=j8u[:])
                nc.vector.tensor_copy(out=s8f[:], in_=s8[:])
                m8 = spool.tile([P, 8], f32, tag="m8")      # member = s - 2*j
                nc.vector.scalar_tensor_tensor(out=m8[:], in0=j8f[:], scalar=-2.0,
                                               in1=s8f[:], op0=AL.mult, op1=AL.add)
                # pairidx by j
                psel = spool.tile([P, 8], f32, tag="psel")
                junk2 = spool.tile([P, NPSEL], f32, tag="junk10")
                for j in range(8):
                    nc.vector.scalar_tensor_tensor(
                        out=junk2[:], in0=io16_sb[:, :NPSEL], scalar=j8f[:, j:j + 1], in1=pj[:],
                        op0=AL.is_equal, op1=AL.mult, accum_out=psel[:, j:j + 1])
                psel_u = spool.tile([P, 8], u32, tag="psel_u")
                wsel_u = spool.tile([P, 8], u32, tag="wsel_u")
                nc.vector.tensor_copy(out=psel_u[:], in_=psel[:])
                nc.vector.tensor_scalar(out=wsel_u[:], in0=psel_u[:], scalar1=10, scalar2=None,
                                        op0=AL.logical_shift_right)
                wself = spool.tile([P, 8], f32, tag="wself")
                nc.vector.tensor_copy(out=wself[:], in_=wsel_u[:])
                # key = psel + 1024*wsel + 1024*m
                keyf = spool.tile([P, 8], f32, tag="keyf")
                nc.vector.scalar_tensor_tensor(out=keyf[:], in0=wself[:], scalar=1024.0,
                                               in1=psel[:], op0=AL.mult, op1=AL.add)
                nc.vector.scalar_tensor_tensor(out=keyf[:], in0=m8[:], scalar=1024.0,
                                               in1=keyf[:], op0=AL.mult, op1=AL.add)
                if dbg:
                    nc.sync.dma_start(out=d_key[:, t * 8:(t + 1) * 8], in_=keyf[:])
                key_u = spool.tile([P, 8], u32, tag="key_u")
                nc.vector.tensor_copy(out=key_u[:], in_=keyf[:])

                # softmax over v8 (descending, v8[0] is max); exp scale 1/SSEL
                w8 = spool.tile([P, 8], f32, tag="w8")
                sm8 = spool.tile([P, 1], f32, tag="sm8")
                nc.vector.tensor_scalar(out=w8[:], in0=v8[:], scalar1=v8[:, :1], scalar2=None,
                                        op0=AL.subtract)
                nc.scalar.activation(out=w8[:], in_=w8[:], func=mybir.ActivationFunctionType.Exp,
                                     scale=float(1.0 / SSEL), accum_out=sm8[:, :1])
                rcp8 = spool.tile([P, 1], f32, tag="rcp8")
                nc.vector.reciprocal(out=rcp8[:], in_=sm8[:, :1])
                nc.vector.tensor_scalar(out=w8[:], in0=w8[:], scalar1=rcp8[:, :1], scalar2=None,
                                        op0=AL.mult)
                if dbg:
                    nc.sync.dma_start(out=d_w8[:, t * 8:(t + 1) * 8], in_=w8[:])

                # ============ G3 + Vsum via diag matmuls ============
                g3 = g3pool.tile([P, 8 * D_MODEL], f16, tag="g3", name=f"g3_{t}")
                for s in range(8):
                    nc.gpsimd.indirect_dma_start(
                        out=g3[:, s * D_MODEL:(s + 1) * D_MODEL], out_offset=None,
                        in_=VD16[:],
                        in_offset=bass.IndirectOffsetOnAxis(ap=key_u[:, s:s + 1], axis=0))
                dg8 = dgpool.tile([P, 8 * P], f16, tag="dg8", name=f"dg8_{t}")
                for s in range(8):
                    nc.vector.tensor_scalar(out=dg8[:, s * P:(s + 1) * P], in0=id_sb[:],
                                            scalar1=w8[:, s:s + 1], scalar2=None, op0=AL.mult)
                accps_full = psw.tile([P, 2048], f32, space="PSUM", tag="w", name=f"accps_{t}")
                accps = accps_full[:, :D_MODEL]
                for h in range(2):
                    for s in range(8):
                        nc.tensor.matmul(
                            out=accps[:, h * 512:(h + 1) * 512],
                            lhsT=dg8[:, s * P:(s + 1) * P],
                            rhs=g3[:, s * D_MODEL + h * 512:s * D_MODEL + (h + 1) * 512],
                            start=(s == 0), stop=(s == 7))
                accf = apool.tile([P, D_MODEL], f32, tag="accf", name=f"accf_{t}")
                nc.scalar.copy(out=accf[:], in_=accps)
                nc.sync.dma_start(out=out[t * P:(t + 1) * P, :], in_=accf[:])
                if dbg:
                    nc.sync.dma_start(out=d_wts[:], in_=wts_sb[:])

            for t in range(N_TILES):
                emit_C(t)
                if t > 0:
                    emit_tail(t - 1)
            emit_tail(N_TILES - 1)

    nc.compile()
    return nc


_NC_CACHE = {}


def _get_nc(dbg=False):
    if dbg not in _NC_CACHE:
        _NC_CACHE[dbg] = _build(dbg)
    return _NC_CACHE[dbg]


def _prep_in_maps(x, router_w, compress_neurons, knowledge_K, knowledge_V):
    x = np.asarray(x, dtype=np.float32).reshape(B * S, D_MODEL)
    rwT = np.asarray(router_w, dtype=np.float32).T          # [1024, 16]
    rw_r = np.ascontiguousarray(
        rwT.reshape(N_DC, P, N_COMPRESS).transpose(1, 0, 2).reshape(P, N_DC * N_COMPRESS))
    cn = np.asarray(compress_neurons, dtype=np.float32)     # [16, 1024, 128]
    # Wg[p, (gh*8+dc)*1024 + ln*128 + r] = cn[8*gh+ln, dc*128+p, r]
    Wg = np.ascontiguousarray(
        cn.reshape(2, 8, N_DC, P, RANK).transpose(3, 0, 2, 1, 4).reshape(P, 2 * N_DC * 1024))
    K = np.asarray(knowledge_K, dtype=np.float32)
    KT16 = np.ascontiguousarray(K.T).astype(np.float16)     # [128, 32768]
    # KP32[w*1024+j] = (K[2048w+j], K[2048w+1024+j]) in f32
    KP32 = np.ascontiguousarray(
        K.reshape(N_WIN, 2, HALF, RANK).transpose(0, 2, 1, 3).reshape(N_PAIR, 2 * RANK))
    V16 = np.asarray(knowledge_V, dtype=np.float32).astype(np.float16)
    ident = np.eye(P, dtype=np.float16)
    idf32 = np.eye(P, dtype=np.float32)
    iotaP = np.arange(P, dtype=np.float32).reshape(P, 1)
    io16 = np.broadcast_to(np.arange(24, dtype=np.float32), (P, 24)).copy()

    in_maps = []
    for c in range(N_CORES):
        xs = x[c * TOK_PER_CORE:(c + 1) * TOK_PER_CORE]
        xTc = np.ascontiguousarray(
            xs.T.reshape(N_DC, P, TOK_PER_CORE).transpose(1, 0, 2).reshape(P, N_DC * TOK_PER_CORE))
        in_maps.append(dict(xT=xTc, rw=rw_r, Wg=Wg, KT16=KT16, KP32=KP32, VD16=V16,
                            ident=ident, idf32=idf32, iotaP=iotaP, io16=io16))
    return in_maps


def _ensure_ntff_hook():
    import sys as _sys
    import types as _types
    if "antenv.axon_hooks" in _sys.modules:
        return
    try:
        import antenv.axon_hooks  # noqa: F401
        return
    except ImportError:
        pass
    mod = _types.ModuleType("antenv.axon_hooks")
    _state = {"hook": None}
    mod.set_axon_ntff_profile_hook = lambda h: _state.__setitem__("hook", h)
    mod.get_axon_ntff_profile_hook = lambda: _state["hook"]
    _sys.modules["antenv.axon_hooks"] = mod
    try:
        from trn_agent_boot.trn_boot import _ntff_profile_via_ctypes
        mod.set_axon_ntff_profile_hook(_ntff_profile_via_ctypes("/opt/axon/libaxon_pjrt.so"))
    except Exception:
        pass


def _run(inputs, trace=False, dbg=False):
    if trace:
        _ensure_ntff_hook()
    nc = _get_nc(dbg)
    in_maps = _prep_in_maps(**inputs)
    res = run_bass_kernel_spmd(nc, in_maps, core_ids=list(range(N_CORES)), trace=trace)
    out = np.concatenate([res.results[c]["out"] for c in range(N_CORES)], axis=0)
    return out.reshape(B, S, D_MODEL).astype(np.float32), res


def kernel(x, router_w, compress_neurons, knowledge_K, knowledge_V):
    out, _ = _run(dict(x=x, router_w=router_w, compress_neurons=compress_neurons,
                       knowledge_K=knowledge_K, knowledge_V=knowledge_V))
    return out


# revision 34
# speedup vs baseline: 1.3108x; 1.0441x over previous
"""NeuronMemory retrieval kernel v4 for 8 TRN2 NeuronCores.

Data-parallel over tokens (512/core, 4 tiles of 128). Per-core, per tile:
  A:  router scores (PE) + softmax -> wts f32 [128,16]
  B:  y = x @ W_n for 16 neurons (PE, f16, 32 matmuls) -> y16 f16 SBUF
      QT[r,tok] = sum_n diag(w_n)-weighted y via 16 PE matmuls (diag as rhs)
      qt16 = f16(QT * SCALE); q16 = transpose(qt16) via PE
  C:  16 windows of 2048 keys: scores = qt16^T @ KT16 (PE, f16) -> PSUM f32
      L1 pair-reduce: sp1[w*1024+j] = max(s[2048w+j], s[2048w+1024+j])
        plan A (Z_A windows): scalar stages whole window f16, DVE TT 2x
        plan B (rest):        scalar stages 2nd half,     DVE TT(PSUM,SBUF)
  tree: sp1 [128,16384] -> blockmax bm [128,256] (64 pairs/block) on DVE
  spill: sp1 -> DRAM rows [128*256, 64] f16 (one 4MB DMA)
  L2: top-8 blocks/token (max8+fi8 on bm)
  G1: gather 8 winning block-rows (64 pair-values each) -> g1 [128,512]
  L3: top-8 pairs/token (max8+fi8 on g1) -> pairidx[8] (global pair ids)
  G2: gather 8 K-pair rows (256 f16) from KP16 -> g2; rescore 16 cands
      exactly on DVE (TT mult + TR add vs q16) -> rsc f32 -> f16
  top8: max8+fi8 on rsc16 -> 8 winners; member bit + key reconstruction;
      softmax(v8) -> w8 f32
  G3: gather 8 V rows (2KB f16) -> g3; Vsum via 8 diag matmuls on PE -> out

Pair rows: KP16[w*1024+j] = (K[2048w+j], K[2048w+1024+j]), both f16.
key = pairidx + 1024*(pairidx>>10) + 1024*member.
"""
import copy as _copy

import numpy as np

import concourse.bacc as bacc
import concourse.bass as bass
import concourse.mybir as mybir
from concourse.tile import TileContext
from concourse.bass_utils import run_bass_kernel_spmd

P = 128
D_MODEL = 1024
RANK = 128
N_COMPRESS = 16
N_KNOWLEDGE = 32768
K_TOP = 8
B, S = 2, 2048
N_CORES = 8
TOK_PER_CORE = (B * S) // N_CORES      # 512
N_TILES = TOK_PER_CORE // P            # 4
N_DC = D_MODEL // P                    # 8
N_WIN = 16                             # score windows per tile
WIN = N_KNOWLEDGE // N_WIN             # 2048 keys per window
HALF = WIN // 2                        # 1024 pairs per window
N_PAIR = N_KNOWLEDGE // 2              # 16384 pairs per tile
BLK = 64                               # pairs per block
N_BLK = N_PAIR // BLK                  # 256 blocks
NBSEL = 9                              # blocks gathered per token (8 + tie slack)
NPSEL = 10                             # pairs rescored per token (8 + tie slack)
NCAND = 2 * NPSEL                      # candidate keys
Z_A = 16                               # plan-A windows per tile (0..16)
SCALE = 1.0 / np.sqrt(np.float32(RANK))
SSEL = 0.5                             # extra selection scale (folded into qt16/q16)

f32 = mybir.dt.float32
f16 = mybir.dt.float16
u32 = mybir.dt.uint32

AL = mybir.AluOpType


def _build(dbg=False):
    nc = bacc.Bacc("TRN2", target_bir_lowering=False, debug=False, num_devices=N_CORES)

    # split-f16 hi/lo pairs: x = xh + xl, W = Wh + Wl (exact to ~2^-22)
    xT = nc.declare_dram_parameter("xT", [P, 2 * N_DC * TOK_PER_CORE], f16, isOutput=False)
    rw = nc.declare_dram_parameter("rw", [P, 2 * N_DC * N_COMPRESS], f16, isOutput=False)
    # Wg[p, hl*16384 + (gh*8+dc)*1024 + ln*128 + r] = cn_hl[8*gh+ln, dc*128+p, r]
    Wg = nc.declare_dram_parameter("Wg", [P, 2 * 2 * N_DC * 1024], f16, isOutput=False)
    KT16 = nc.declare_dram_parameter("KT16", [P, N_KNOWLEDGE], f16, isOutput=False)
    KP32 = nc.declare_dram_parameter("KP32", [N_PAIR, 2 * RANK], f32, isOutput=False)
    VD16 = nc.declare_dram_parameter("VD16", [N_KNOWLEDGE, D_MODEL], f16, isOutput=False)
    ident = nc.declare_dram_parameter("ident", [P, P], f16, isOutput=False)
    idf32 = nc.declare_dram_parameter("idf32", [P, P], f32, isOutput=False)
    iotaP = nc.declare_dram_parameter("iotaP", [P, 1], f32, isOutput=False)
    io16 = nc.declare_dram_parameter("io16", [P, 24], f32, isOutput=False)
    out = nc.declare_dram_parameter("out", [TOK_PER_CORE, D_MODEL], f32, isOutput=True)
    if dbg:
        d_wts = nc.declare_dram_parameter("d_wts", [P, N_TILES * N_COMPRESS], f32, isOutput=True)
        d_qt = nc.declare_dram_parameter("d_qt", [P, N_TILES * P], f16, isOutput=True)
        d_bm = nc.declare_dram_parameter("d_bm", [P, N_TILES * N_BLK], f16, isOutput=True)
        d_g1 = nc.declare_dram_parameter("d_g1", [P, N_TILES * 512], f16, isOutput=True)
        d_pj = nc.declare_dram_parameter("d_pj", [P, N_TILES * NPSEL], f32, isOutput=True)
        d_rs = nc.declare_dram_parameter("d_rs", [P, N_TILES * NCAND], f32, isOutput=True)
        d_key = nc.declare_dram_parameter("d_key", [P, N_TILES * 8], f32, isOutput=True)
        d_w8 = nc.declare_dram_parameter("d_w8", [P, N_TILES * 8], f32, isOutput=True)

    sp1D = {t: nc.dram_tensor(f"sp1D_{t}", [P * N_BLK, BLK], f16) for t in range(N_TILES)}

    with TileContext(nc) as tc:
        with (
            tc.tile_pool(name="const", bufs=1) as cpool,
            tc.tile_pool(name="sp1p", bufs=1) as sp1pool,
            tc.tile_pool(name="stg", bufs=2) as stgpool,
            tc.tile_pool(name="wgs", bufs=2) as wgpool,
            tc.tile_pool(name="dgp", bufs=2) as dgpool,
            tc.tile_pool(name="qp", bufs=4) as qpool,
            tc.tile_pool(name="g1p", bufs=2) as g1pool,
            tc.tile_pool(name="g2p", bufs=1) as g2pool,
            tc.tile_pool(name="g3p", bufs=2) as g3pool,
            tc.tile_pool(name="pr", bufs=1) as prpool,
            tc.tile_pool(name="acc", bufs=2) as apool,
            tc.tile_pool(name="sm", bufs=2) as spool,
            tc.tile_pool(name="psw", bufs=2, space="PSUM") as psw,
        ):
            # ---------------- persistent loads ----------------
            xT_sb = cpool.tile([P, 2 * N_DC * TOK_PER_CORE], f16)
            rw_sb = cpool.tile([P, 2 * N_DC * N_COMPRESS], f16)
            kt_sb = cpool.tile([P, N_KNOWLEDGE], f16)
            id_sb = cpool.tile([P, P], f16)
            idf_sb = cpool.tile([P, P], f32)
            iota_sb = cpool.tile([P, 1], f32)
            io16_sb = cpool.tile([P, 24], f32)
            nc.sync.dma_start(out=xT_sb[:], in_=xT[:])
            nc.sync.dma_start(out=rw_sb[:], in_=rw[:])
            nc.sync.dma_start(out=idf_sb[:], in_=idf32[:])
            for q in range(4):
                nc.sync.dma_start(out=kt_sb[:, q * 8192:(q + 1) * 8192],
                                  in_=KT16[:, q * 8192:(q + 1) * 8192])
            nc.sync.dma_start(out=id_sb[:], in_=ident[:])
            nc.sync.dma_start(out=iota_sb[:], in_=iotaP[:])
            nc.sync.dma_start(out=io16_sb[:], in_=io16[:])

            wts_sb = cpool.tile([P, N_TILES * N_COMPRESS], f32)

            def tok(t):
                return slice(t * P, (t + 1) * P)

            # ============ A: router softmax (split-bf16 3-pass) ============
            def xsl(hl, dc, t):
                base = hl * N_DC * TOK_PER_CORE
                return xT_sb[:, base + dc * TOK_PER_CORE + t * P:
                             base + dc * TOK_PER_CORE + (t + 1) * P]

            for t in range(N_TILES):
                rps_full = psw.tile([P, 2048], f32, space="PSUM", tag="w", name=f"rps_{t}")
                rps = rps_full[:, :N_COMPRESS]
                NCC = N_COMPRESS
                for dc in range(N_DC):
                    for k, (hl, rl) in enumerate(((0, 0), (0, 1), (1, 0))):
                        nc.tensor.matmul(
                            out=rps, lhsT=xsl(hl, dc, t),
                            rhs=rw_sb[:, (rl * N_DC + dc) * NCC:(rl * N_DC + dc + 1) * NCC],
                            start=(dc == 0 and k == 0), stop=(dc == N_DC - 1 and k == 2))
                w_ = wts_sb[:, t * N_COMPRESS:(t + 1) * N_COMPRESS]
                mx = spool.tile([P, 1], f32, tag="mx")
                sm = spool.tile([P, 1], f32, tag="sm")
                ex = spool.tile([P, N_COMPRESS], f32, tag="ex")
                nc.vector.tensor_reduce(out=mx[:], in_=rps, op=AL.max, axis=mybir.AxisListType.X)
                nc.vector.tensor_scalar(out=ex[:], in0=rps, scalar1=mx[:, :1], scalar2=None,
                                        op0=AL.subtract)
                nc.scalar.activation(out=ex[:], in_=ex[:], func=mybir.ActivationFunctionType.Exp,
                                     accum_out=sm[:, :1])
                rcp = spool.tile([P, 1], f32, tag="rcp")
                nc.vector.reciprocal(out=rcp[:], in_=sm[:, :1])
                nc.vector.tensor_scalar(out=w_, in0=ex[:], scalar1=rcp[:, :1], scalar2=None,
                                        op0=AL.mult)

            # ============ B (per tile-pair): y via split-bf16, Q combine ============
            Q_sb = cpool.tile([P, N_TILES * P], f32)        # exact Q per tile
            TS = {t: {} for t in range(N_TILES)}

            def emit_B(pr):
                tiles = (2 * pr, 2 * pr + 1)
                for gh in range(2):
                    yps = psw.tile([P, 2048], f32, space="PSUM", tag="w", name=f"yps_{pr}_{gh}")
                    for dc in range(N_DC):
                        whb = wgpool.tile([P, 1024], f16, tag="whb", name=f"wh_{pr}_{gh}_{dc}")
                        wlb = wgpool.tile([P, 1024], f16, tag="wlb", name=f"wl_{pr}_{gh}_{dc}")
                        off = (gh * N_DC + dc) * 1024
                        nc.sync.dma_start(out=whb[:], in_=Wg[:, off:off + 1024])
                        nc.sync.dma_start(out=wlb[:], in_=Wg[:, 16384 + off:16384 + off + 1024])
                        for ti, t in enumerate(tiles):
                            for k, (hl, wb) in enumerate(((0, whb), (0, wlb), (1, whb))):
                                for g2_ in range(2):
                                    nc.tensor.matmul(
                                        out=yps[:, ti * 1024 + g2_ * 512:
                                                ti * 1024 + (g2_ + 1) * 512],
                                        lhsT=xsl(hl, dc, t),
                                        rhs=wb[:, g2_ * 512:(g2_ + 1) * 512],
                                        start=(dc == 0 and k == 0),
                                        stop=(dc == N_DC - 1 and k == 2))
                    for ti, t in enumerate(tiles):
                        q_ = Q_sb[:, t * P:(t + 1) * P]
                        for ln in range(8):
                            n = gh * 8 + ln
                            wcol = wts_sb[:, t * N_COMPRESS + n:t * N_COMPRESS + n + 1]
                            ypart = yps[:, ti * 1024 + ln * P:ti * 1024 + (ln + 1) * P]
                            if gh == 0 and ln == 0:
                                nc.vector.tensor_scalar(out=q_, in0=ypart, scalar1=wcol,
                                                        scalar2=None, op0=AL.mult)
                            else:
                                nc.vector.scalar_tensor_tensor(out=q_, in0=ypart, scalar=wcol,
                                                               in1=q_, op0=AL.mult, op1=AL.add)
                # qt16/qf32 for the pair
                for t in tiles:
                    qtps_full = psw.tile([P, 2048], f32, space="PSUM", tag="w", name=f"qtps_{t}")
                    qtps = qtps_full[:, :P]
                    nc.tensor.transpose(out=qtps, in_=Q_sb[:, tok(t)], identity=idf_sb[:])
                    qt16 = qpool.tile([P, P], f16, tag="qt16", name=f"qt16_{t}")
                    nc.scalar.activation(out=qt16[:], in_=qtps,
                                         func=mybir.ActivationFunctionType.Copy,
                                         scale=float(SCALE * SSEL))
                    if dbg:
                        nc.sync.dma_start(out=d_qt[:, tok(t)], in_=qt16[:])
                    qf32 = qpool.tile([P, P], f32, tag="qf32", name=f"qf32_{t}")
                    nc.scalar.activation(out=qf32[:], in_=Q_sb[:, tok(t)],
                                         func=mybir.ActivationFunctionType.Copy,
                                         scale=float(SCALE * SSEL))
                    TS[t].update(qt16=qt16, qf32=qf32)

            def emit_C(t):
                qt16 = TS[t]["qt16"]
                # ============ C: scores + L1 pair-reduce (per half-tile) ============
                bm = dgpool.tile([P, N_BLK], f16, tag="bm", name=f"bm_{t}")
                TS[t]["bm"] = bm
                NBH = N_BLK // 2                         # 128 blocks per half
                for h in range(2):
                    sp1 = sp1pool.tile([P, N_PAIR // 2], f16, tag="sp1", name=f"sp1_{t}_{h}")
                    for wl in range(N_WIN // 2):
                        w = h * (N_WIN // 2) + wl
                        wps = psw.tile([P, WIN], f32, space="PSUM", tag="w", name=f"wps_{t}_{w}")
                        for j in range(4):
                            nc.tensor.matmul(
                                out=wps[:, j * 512:(j + 1) * 512],
                                lhsT=qt16[:],
                                rhs=kt_sb[:, w * WIN + j * 512:w * WIN + (j + 1) * 512],
                                start=True, stop=True)
                        sp1w = sp1[:, wl * HALF:(wl + 1) * HALF]
                        if wl < Z_A // 2:
                            stg = stgpool.tile([P, WIN], f16, tag="stg", name=f"stgA_{t}_{w}")
                            nc.scalar.copy(out=stg[:], in_=wps[:])
                            nc.vector.tensor_tensor(out=sp1w, in0=stg[:, :HALF],
                                                    in1=stg[:, HALF:], op=AL.max)
                        else:
                            stg = stgpool.tile([P, WIN], f16, tag="stg", name=f"stgB_{t}_{w}")
                            nc.scalar.copy(out=stg[:, :HALF], in_=wps[:, HALF:])
                            nc.vector.tensor_tensor(out=sp1w, in0=wps[:, :HALF],
                                                    in1=stg[:, :HALF], op=AL.max)

                    # spill this half: DRAM rows p*256 + h*128 + local
                    o_sp = _copy.copy(sp1D[t][:])
                    o_sp.offset = h * NBH * BLK
                    o_sp.ap = mybir.VecI64Pair([[N_BLK * BLK, P], [1, NBH * BLK]])
                    nc.sync.dma_start(out=o_sp, in_=sp1[:])

                    # tree: blockmax for this half's 128 blocks
                    tw = sp1
                    for wd in (32, 16, 8, 4, 2):
                        tag = "trA" if wd in (32, 8, 2) else "trB"
                        nxt = sp1pool.tile([P, NBH * wd], f16, tag=tag, name=f"tr{wd}_{t}_{h}")
                        s3 = tw[:].rearrange("p (b w) -> p b w", b=NBH)
                        nc.vector.tensor_tensor(out=nxt[:].rearrange("p (b w) -> p b w", b=NBH),
                                                in0=s3[:, :, 0:wd], in1=s3[:, :, wd:2 * wd],
                                                op=AL.max)
                        tw = nxt
                    s3 = tw[:].rearrange("p (b w) -> p b w", b=NBH)
                    nc.vector.tensor_tensor(
                        out=bm[:, h * NBH:(h + 1) * NBH].rearrange("p (b w) -> p b w", b=NBH),
                        in0=s3[:, :, 0:1], in1=s3[:, :, 1:2], op=AL.max)
                if dbg:
                    nc.sync.dma_start(out=d_bm[:, t * N_BLK:(t + 1) * N_BLK], in_=bm[:])

            def emit_tail(t):
                bm = TS[t]["bm"]
                qf32 = TS[t]["qf32"]
                # ============ L2: top-9 blocks (8 + tie slack) ============
                bv8 = spool.tile([P, 8], f16, tag="bv8")
                bu8 = spool.tile([P, 8], u32, tag="bu8")
                nc.vector.max(out=bv8[:], in_=bm[:])
                nc.vector.max_index(out=bu8[:], in_max=bv8[:], in_values=bm[:])
                bmr = spool.tile([P, N_BLK], f16, tag="bmr")
                nc.vector.match_replace(out=bmr[:], in_to_replace=bv8[:], in_values=bm[:],
                                        imm_value=-60000.0)
                bv9 = spool.tile([P, 8], f16, tag="bv9")
                bu9 = spool.tile([P, 8], u32, tag="bu9")
                nc.vector.max(out=bv9[:], in_=bmr[:])
                nc.vector.max_index(out=bu9[:], in_max=bv9[:], in_values=bmr[:])
                bif = spool.tile([P, NBSEL], f32, tag="bif")
                nc.vector.tensor_copy(out=bif[:, :8], in_=bu8[:])
                nc.vector.tensor_copy(out=bif[:, 8:NBSEL], in_=bu9[:, :NBSEL - 8])
                rowb = spool.tile([P, 1], f32, tag="rowb")
                nc.vector.tensor_scalar(out=rowb[:], in0=iota_sb[:], scalar1=float(N_BLK),
                                        scalar2=None, op0=AL.mult)
                gidx = spool.tile([P, NBSEL], f32, tag="gidx")
                nc.vector.tensor_scalar(out=gidx[:], in0=bif[:], scalar1=rowb[:, :1],
                                        scalar2=None, op0=AL.add)
                gidx_u = spool.tile([P, NBSEL], u32, tag="gidx_u")
                nc.vector.tensor_copy(out=gidx_u[:], in_=gidx[:])

                # ============ G1: gather winning blocks ============
                g1 = g1pool.tile([P, NBSEL * BLK], f16, tag="g1", name=f"g1_{t}")
                for s in range(NBSEL):
                    nc.gpsimd.indirect_dma_start(
                        out=g1[:, s * BLK:(s + 1) * BLK], out_offset=None,
                        in_=sp1D[t][:],
                        in_offset=bass.IndirectOffsetOnAxis(ap=gidx_u[:, s:s + 1], axis=0))
                if dbg:
                    nc.sync.dma_start(out=d_g1[:, t * 512:(t + 1) * 512], in_=g1[:, :512])

                # ============ L3: top-10 pairs (8 + tie slack) ============
                pv8 = spool.tile([P, 8], f16, tag="pv8")
                pp8 = spool.tile([P, 8], u32, tag="pp8")
                nc.vector.max(out=pv8[:], in_=g1[:])
                nc.vector.max_index(out=pp8[:], in_max=pv8[:], in_values=g1[:])
                g1r = g1pool.tile([P, NBSEL * BLK], f16, tag="g1r", name=f"g1r_{t}")
                nc.vector.match_replace(out=g1r[:], in_to_replace=pv8[:], in_values=g1[:],
                                        imm_value=-60000.0)
                pv9 = spool.tile([P, 8], f16, tag="pv9")
                pp9 = spool.tile([P, 8], u32, tag="pp9")
                nc.vector.max(out=pv9[:], in_=g1r[:])
                nc.vector.max_index(out=pp9[:], in_max=pv9[:], in_values=g1r[:])
                ppN = spool.tile([P, NPSEL], u32, tag="ppN")
                nc.vector.tensor_copy(out=ppN[:, :8], in_=pp8[:])
                nc.vector.tensor_copy(out=ppN[:, 8:NPSEL], in_=pp9[:, :NPSEL - 8])
                slot_u = spool.tile([P, NPSEL], u32, tag="slot_u")
                nc.vector.tensor_scalar(out=slot_u[:], in0=ppN[:], scalar1=6, scalar2=None,
                                        op0=AL.logical_shift_right)
                slotf = spool.tile([P, NPSEL], f32, tag="slotf")
                ppNf = spool.tile([P, NPSEL], f32, tag="ppNf")
                nc.vector.tensor_copy(out=slotf[:], in_=slot_u[:])
                nc.vector.tensor_copy(out=ppNf[:], in_=ppN[:])
                # off = pp - 64*slot
                offN = spool.tile([P, NPSEL], f32, tag="offN")
                nc.vector.scalar_tensor_tensor(out=offN[:], in0=slotf[:], scalar=-float(BLK),
                                               in1=ppNf[:], op0=AL.mult, op1=AL.add)
                # blk by slot: is_equal-accum over NBSEL slots
                blkj = spool.tile([P, NPSEL], f32, tag="blkj")
                junk = spool.tile([P, NBSEL], f32, tag="junk9")
                for j in range(NPSEL):
                    nc.vector.scalar_tensor_tensor(
                        out=junk[:], in0=io16_sb[:, :NBSEL], scalar=slotf[:, j:j + 1],
                        in1=bif[:], op0=AL.is_equal, op1=AL.mult,
                        accum_out=blkj[:, j:j + 1])
                # pairidx = blk*64 + off
                pj = spool.tile([P, NPSEL], f32, tag="pj")
                nc.vector.scalar_tensor_tensor(out=pj[:], in0=blkj[:], scalar=float(BLK),
                                               in1=offN[:], op0=AL.mult, op1=AL.add)
                if dbg:
                    nc.sync.dma_start(out=d_pj[:, t * NPSEL:(t + 1) * NPSEL], in_=pj[:])
                pj_u = spool.tile([P, NPSEL], u32, tag="pj_u")
                nc.vector.tensor_copy(out=pj_u[:], in_=pj[:])

                # ============ G2 + exact rescore (f32) ============
                g2 = g2pool.tile([P, NPSEL * 2 * RANK], f32, tag="g2", name=f"g2_{t}")
                for s in range(NPSEL):
                    nc.gpsimd.indirect_dma_start(
                        out=g2[:, s * 256:(s + 1) * 256], out_offset=None,
                        in_=KP32[:],
                        in_offset=bass.IndirectOffsetOnAxis(ap=pj_u[:, s:s + 1], axis=0))
                rsc = spool.tile([P, NCAND], f32, tag="rsc")
                HC = NCAND // 2
                for hc in range(2):
                    prod = prpool.tile([P, HC * RANK], f32, tag="prod", name=f"prod_{t}_{hc}")
                    qb = _copy.copy(qf32[:])
                    qb.ap = mybir.VecI64Pair([[qb.ap[0][0], P], [0, HC], [1, RANK]])
                    nc.vector.tensor_tensor(
                        out=prod[:].rearrange("p (c r) -> p c r", c=HC),
                        in0=g2[:, hc * HC * RANK:(hc + 1) * HC * RANK]
                            .rearrange("p (c r) -> p c r", c=HC),
                        in1=qb, op=AL.mult)
                    nc.vector.tensor_reduce(out=rsc[:, hc * HC:(hc + 1) * HC],
                                            in_=prod[:].rearrange("p (c r) -> p c r", c=HC),
                                            op=AL.add, axis=mybir.AxisListType.X)
                if dbg:
                    nc.sync.dma_start(out=d_rs[:, t * NCAND:(t + 1) * NCAND], in_=rsc[:])

                # ============ exact top-8 of NCAND (f32) + keys ============
                v8 = spool.tile([P, 8], f32, tag="v8")
                s8 = spool.tile([P, 8], u32, tag="s8")
                nc.vector.max(out=v8[:], in_=rsc[:])
                nc.vector.max_index(out=s8[:], in_max=v8[:], in_values=rsc[:])
                j8u = spool.tile([P, 8], u32, tag="j8u")
                nc.vector.tensor_scalar(out=j8u[:], in0=s8[:], scalar1=1, scalar2=None,
                                        op0=AL.logical_shift_right)
                j8f = spool.tile([P, 8], f32, tag="j8f")
                s8f = spool.tile([P, 8], f32, tag="s8f")
                nc.vector.tensor_copy(out=j8f[:], in_=j8u[:])
                nc.vector.tensor_copy(out=s8f[:], in_=s8[:])
                m8 = spool.tile([P, 8], f32, tag="m8")      # member = s - 2*j
                nc.vector.scalar_tensor_tensor(out=m8[:], in0=j8f[:], scalar=-2.0,
                                               in1=s8f[:], op0=AL.mult, op1=AL.add)
                # pairidx by j
                psel = spool.tile([P, 8], f32, tag="psel")
                junk2 = spool.tile([P, NPSEL], f32, tag="junk10")
                for j in range(8):
                    nc.vector.scalar_tensor_tensor(
                        out=junk2[:], in0=io16_sb[:, :NPSEL], scalar=j8f[:, j:j + 1], in1=pj[:],
                        op0=AL.is_equal, op1=AL.mult, accum_out=psel[:, j:j + 1])
                psel_u = spool.tile([P, 8], u32, tag="psel_u")
                wsel_u = spool.tile([P, 8], u32, tag="wsel_u")
                nc.vector.tensor_copy(out=psel_u[:], in_=psel[:])
                nc.vector.tensor_scalar(out=wsel_u[:], in0=psel_u[:], scalar1=10, scalar2=None,
                                        op0=AL.logical_shift_right)
                wself = spool.tile([P, 8], f32, tag="wself")
                nc.vector.tensor_copy(out=wself[:], in_=wsel_u[:])
                # key = psel + 1024*wsel + 1024*m
                keyf = spool.tile([P, 8], f32, tag="keyf")
                nc.vector.scalar_tensor_tensor(out=keyf[:], in0=wself[:], scalar=1024.0,
                                               in1=psel[:], op0=AL.mult, op1=AL.add)
                nc.vector.scalar_tensor_tensor(out=keyf[:], in0=m8[:], scalar=1024.0,
                                               in1=keyf[:], op0=AL.mult, op1=AL.add)
                if dbg:
                    nc.sync.dma_start(out=d_key[:, t * 8:(t + 1) * 8], in_=keyf[:])
                key_u = spool.tile([P, 8], u32, tag="key_u")
                nc.vector.tensor_copy(out=key_u[:], in_=keyf[:])

                # softmax over v8 (descending, v8[0] is max); exp scale 1/SSEL
                w8 = spool.tile([P, 8], f32, tag="w8")
                sm8 = spool.tile([P, 1], f32, tag="sm8")
                nc.vector.tensor_scalar(out=w8[:], in0=v8[:], scalar1=v8[:, :1], scalar2=None,
                                        op0=AL.subtract)
                nc.scalar.activation(out=w8[:], in_=w8[:], func=mybir.ActivationFunctionType.Exp,
                                     scale=float(1.0 / SSEL), accum_out=sm8[:, :1])
                rcp8 = spool.tile([P, 1], f32, tag="rcp8")
                nc.vector.reciprocal(out=rcp8[:], in_=sm8[:, :1])
                nc.vector.tensor_scalar(out=w8[:], in0=w8[:], scalar1=rcp8[:, :1], scalar2=None,
                                        op0=AL.mult)
                if dbg:
                    nc.sync.dma_start(out=d_w8[:, t * 8:(t + 1) * 8], in_=w8[:])

                # ============ G3 + Vsum via diag matmuls ============
                g3 = g3pool.tile([P, 8 * D_MODEL], f16, tag="g3", name=f"g3_{t}")
                for s in range(8):
                    nc.gpsimd.indirect_dma_start(
                        out=g3[:, s * D_MODEL:(s + 1) * D_MODEL], out_offset=None,
                        in_=VD16[:],
                        in_offset=bass.IndirectOffsetOnAxis(ap=key_u[:, s:s + 1], axis=0))
                dg8 = dgpool.tile([P, 8 * P], f16, tag="dg8", name=f"dg8_{t}")
                for s in range(8):
                    nc.vector.tensor_scalar(out=dg8[:, s * P:(s + 1) * P], in0=id_sb[:],
                                            scalar1=w8[:, s:s + 1], scalar2=None, op0=AL.mult)
                accps_full = psw.tile([P, 2048], f32, space="PSUM", tag="w", name=f"accps_{t}")
                accps = accps_full[:, :D_MODEL]
                for h in range(2):
                    for s in range(8):
                        nc.tensor.matmul(
                            out=accps[:, h * 512:(h + 1) * 512],
                            lhsT=dg8[:, s * P:(s + 1) * P],
                            rhs=g3[:, s * D_MODEL + h * 512:s * D_MODEL + (h + 1) * 512],
                            start=(s == 0), stop=(s == 7))
                accf = apool.tile([P, D_MODEL], f32, tag="accf", name=f"accf_{t}")
                nc.scalar.copy(out=accf[:], in_=accps)
                nc.sync.dma_start(out=out[t * P:(t + 1) * P, :], in_=accf[:])
                if dbg:
                    nc.sync.dma_start(out=d_wts[:], in_=wts_sb[:])

            emit_B(0)
            emit_C(0)
            emit_B(1)
            emit_C(1)
            emit_tail(0)
            emit_C(2)
            emit_tail(1)
            emit_C(3)
            emit_tail(2)
            emit_tail(3)

    nc.compile()
    return nc


_NC_CACHE = {}


def _get_nc(dbg=False):
    if dbg not in _NC_CACHE:
        _NC_CACHE[dbg] = _build(dbg)
    return _NC_CACHE[dbg]


def _split16(a):
    hi = a.astype(np.float16)
    lo = (a - hi.astype(np.float32)).astype(np.float16)
    return hi, lo


def _prep_in_maps(x, router_w, compress_neurons, knowledge_K, knowledge_V):
    x = np.asarray(x, dtype=np.float32).reshape(B * S, D_MODEL)
    rwT = np.asarray(router_w, dtype=np.float32).T          # [1024, 16]
    rw_f = np.ascontiguousarray(
        rwT.reshape(N_DC, P, N_COMPRESS).transpose(1, 0, 2).reshape(P, N_DC * N_COMPRESS))
    rw_r = np.concatenate(_split16(rw_f), axis=1)           # [P, 2*8*16]
    cn = np.asarray(compress_neurons, dtype=np.float32)     # [16, 1024, 128]
    # Wg[p, hl*16384 + (gh*8+dc)*1024 + ln*128 + r] = cn_hl[8*gh+ln, dc*128+p, r]
    Wg_f = np.ascontiguousarray(
        cn.reshape(2, 8, N_DC, P, RANK).transpose(3, 0, 2, 1, 4).reshape(P, 2 * N_DC * 1024))
    Wg = np.concatenate(_split16(Wg_f), axis=1)             # [P, 32768]
    K = np.asarray(knowledge_K, dtype=np.float32)
    KT16 = np.ascontiguousarray(K.T).astype(np.float16)     # [128, 32768]
    # KP32[w*1024+j] = (K[2048w+j], K[2048w+1024+j]) in f32
    KP32 = np.ascontiguousarray(
        K.reshape(N_WIN, 2, HALF, RANK).transpose(0, 2, 1, 3).reshape(N_PAIR, 2 * RANK))
    V16 = np.asarray(knowledge_V, dtype=np.float32).astype(np.float16)
    ident = np.eye(P, dtype=np.float16)
    idf32 = np.eye(P, dtype=np.float32)
    iotaP = np.arange(P, dtype=np.float32).reshape(P, 1)
    io16 = np.broadcast_to(np.arange(24, dtype=np.float32), (P, 24)).copy()

    in_maps = []
    for c in range(N_CORES):
        xs = x[c * TOK_PER_CORE:(c + 1) * TOK_PER_CORE]
        xTf = np.ascontiguousarray(
            xs.T.reshape(N_DC, P, TOK_PER_CORE).transpose(1, 0, 2).reshape(P, N_DC * TOK_PER_CORE))
        xTc = np.concatenate(_split16(xTf), axis=1)
        in_maps.append(dict(xT=xTc, rw=rw_r, Wg=Wg, KT16=KT16, KP32=KP32, VD16=V16,
                            ident=ident, idf32=idf32, iotaP=iotaP, io16=io16))
    return in_maps


def _ensure_ntff_hook():
    import sys as _sys
    import types as _types
    if "antenv.axon_hooks" in _sys.modules:
        return
    try:
        import antenv.axon_hooks  # noqa: F401
        return
    except ImportError:
        pass
    mod = _types.ModuleType("antenv.axon_hooks")
    _state = {"hook": None}
    mod.set_axon_ntff_profile_hook = lambda h: _state.__setitem__("hook", h)
    mod.get_axon_ntff_profile_hook = lambda: _state["hook"]
    _sys.modules["antenv.axon_hooks"] = mod
    try:
        from trn_agent_boot.trn_boot import _ntff_profile_via_ctypes
        mod.set_axon_ntff_profile_hook(_ntff_profile_via_ctypes("/opt/axon/libaxon_pjrt.so"))
    except Exception:
        pass


def _run(inputs, trace=False, dbg=False):
    if trace:
        _ensure_ntff_hook()
    nc = _get_nc(dbg)
    in_maps = _prep_in_maps(**inputs)
    res = run_bass_kernel_spmd(nc, in_maps, core_ids=list(range(N_CORES)), trace=trace)
    out = np.concatenate([res.results[c]["out"] for c in range(N_CORES)], axis=0)
    return out.reshape(B, S, D_MODEL).astype(np.float32), res


def kernel(x, router_w, compress_neurons, knowledge_K, knowledge_V):
    out, _ = _run(dict(x=x, router_w=router_w, compress_neurons=compress_neurons,
                       knowledge_K=knowledge_K, knowledge_V=knowledge_V))
    return out


# revision 37
# speedup vs baseline: 1.3122x; 1.0011x over previous
"""NeuronMemory retrieval kernel v4 for 8 TRN2 NeuronCores.

Data-parallel over tokens (512/core, 4 tiles of 128). Per-core, per tile:
  A:  router scores (PE) + softmax -> wts f32 [128,16]
  B:  y = x @ W_n for 16 neurons (PE, f16, 32 matmuls) -> y16 f16 SBUF
      QT[r,tok] = sum_n diag(w_n)-weighted y via 16 PE matmuls (diag as rhs)
      qt16 = f16(QT * SCALE); q16 = transpose(qt16) via PE
  C:  16 windows of 2048 keys: scores = qt16^T @ KT16 (PE, f16) -> PSUM f32
      L1 pair-reduce: sp1[w*1024+j] = max(s[2048w+j], s[2048w+1024+j])
        plan A (Z_A windows): scalar stages whole window f16, DVE TT 2x
        plan B (rest):        scalar stages 2nd half,     DVE TT(PSUM,SBUF)
  tree: sp1 [128,16384] -> blockmax bm [128,256] (64 pairs/block) on DVE
  spill: sp1 -> DRAM rows [128*256, 64] f16 (one 4MB DMA)
  L2: top-8 blocks/token (max8+fi8 on bm)
  G1: gather 8 winning block-rows (64 pair-values each) -> g1 [128,512]
  L3: top-8 pairs/token (max8+fi8 on g1) -> pairidx[8] (global pair ids)
  G2: gather 8 K-pair rows (256 f16) from KP16 -> g2; rescore 16 cands
      exactly on DVE (TT mult + TR add vs q16) -> rsc f32 -> f16
  top8: max8+fi8 on rsc16 -> 8 winners; member bit + key reconstruction;
      softmax(v8) -> w8 f32
  G3: gather 8 V rows (2KB f16) -> g3; Vsum via 8 diag matmuls on PE -> out

Pair rows: KP16[w*1024+j] = (K[2048w+j], K[2048w+1024+j]), both f16.
key = pairidx + 1024*(pairidx>>10) + 1024*member.
"""
import copy as _copy

import numpy as np

import concourse.bacc as bacc
import concourse.bass as bass
import concourse.mybir as mybir
from concourse.tile import TileContext
from concourse.bass_utils import run_bass_kernel_spmd

P = 128
D_MODEL = 1024
RANK = 128
N_COMPRESS = 16
N_KNOWLEDGE = 32768
K_TOP = 8
B, S = 2, 2048
N_CORES = 8
TOK_PER_CORE = (B * S) // N_CORES      # 512
N_TILES = TOK_PER_CORE // P            # 4
N_DC = D_MODEL // P                    # 8
N_WIN = 16                             # score windows per tile
WIN = N_KNOWLEDGE // N_WIN             # 2048 keys per window
HALF = WIN // 2                        # 1024 pairs per window
N_PAIR = N_KNOWLEDGE // 2              # 16384 pairs per tile
BLK = 64                               # pairs per block
N_BLK = N_PAIR // BLK                  # 256 blocks
NBSEL = 9                              # blocks gathered per token (8 + tie slack)
NPSEL = 10                             # pairs rescored per token (8 + tie slack)
NCAND = 2 * NPSEL                      # candidate keys
Z_A = 4                                # plan-A windows per tile (0..16)
SCALE = 1.0 / np.sqrt(np.float32(RANK))
SSEL = 0.5                             # extra selection scale (folded into qt16/q16)

f32 = mybir.dt.float32
f16 = mybir.dt.float16
u32 = mybir.dt.uint32

AL = mybir.AluOpType


def _build(dbg=False):
    nc = bacc.Bacc("TRN2", target_bir_lowering=False, debug=False, num_devices=N_CORES)

    # split-f16 hi/lo pairs: x = xh + xl, W = Wh + Wl (exact to ~2^-22)
    xT = nc.declare_dram_parameter("xT", [P, 2 * N_DC * TOK_PER_CORE], f16, isOutput=False)
    rw = nc.declare_dram_parameter("rw", [P, 2 * N_DC * N_COMPRESS], f16, isOutput=False)
    # Wg[p, hl*16384 + (gh*8+dc)*1024 + ln*128 + r] = cn_hl[8*gh+ln, dc*128+p, r]
    Wg = nc.declare_dram_parameter("Wg", [P, 2 * 2 * N_DC * 1024], f16, isOutput=False)
    KT16 = nc.declare_dram_parameter("KT16", [P, N_KNOWLEDGE], f16, isOutput=False)
    KP32 = nc.declare_dram_parameter("KP32", [N_PAIR, 2 * RANK], f32, isOutput=False)
    VD16 = nc.declare_dram_parameter("VD16", [N_KNOWLEDGE, D_MODEL], f16, isOutput=False)
    ident = nc.declare_dram_parameter("ident", [P, P], f16, isOutput=False)
    idf32 = nc.declare_dram_parameter("idf32", [P, P], f32, isOutput=False)
    iotaP = nc.declare_dram_parameter("iotaP", [P, 1], f32, isOutput=False)
    io16 = nc.declare_dram_parameter("io16", [P, 24], f32, isOutput=False)
    out = nc.declare_dram_parameter("out", [TOK_PER_CORE, D_MODEL], f32, isOutput=True)
    if dbg:
        d_wts = nc.declare_dram_parameter("d_wts", [P, N_TILES * N_COMPRESS], f32, isOutput=True)
        d_qt = nc.declare_dram_parameter("d_qt", [P, N_TILES * P], f16, isOutput=True)
        d_bm = nc.declare_dram_parameter("d_bm", [P, N_TILES * N_BLK], f16, isOutput=True)
        d_g1 = nc.declare_dram_parameter("d_g1", [P, N_TILES * 512], f16, isOutput=True)
        d_pj = nc.declare_dram_parameter("d_pj", [P, N_TILES * NPSEL], f32, isOutput=True)
        d_rs = nc.declare_dram_parameter("d_rs", [P, N_TILES * NCAND], f32, isOutput=True)
        d_key = nc.declare_dram_parameter("d_key", [P, N_TILES * 8], f32, isOutput=True)
        d_w8 = nc.declare_dram_parameter("d_w8", [P, N_TILES * 8], f32, isOutput=True)

    sp1D = {t: nc.dram_tensor(f"sp1D_{t}", [P * N_BLK, BLK], f16) for t in range(N_TILES)}

    with TileContext(nc) as tc:
        with (
            tc.tile_pool(name="const", bufs=1) as cpool,
            tc.tile_pool(name="sp1p", bufs=1) as sp1pool,
            tc.tile_pool(name="stg", bufs=2) as stgpool,
            tc.tile_pool(name="wgs", bufs=2) as wgpool,
            tc.tile_pool(name="dgp", bufs=2) as dgpool,
            tc.tile_pool(name="qp", bufs=4) as qpool,
            tc.tile_pool(name="g1p", bufs=2) as g1pool,
            tc.tile_pool(name="g2p", bufs=1) as g2pool,
            tc.tile_pool(name="g3p", bufs=2) as g3pool,
            tc.tile_pool(name="pr", bufs=1) as prpool,
            tc.tile_pool(name="acc", bufs=2) as apool,
            tc.tile_pool(name="sm", bufs=2) as spool,
            tc.tile_pool(name="psw", bufs=2, space="PSUM") as psw,
        ):
            # ---------------- persistent loads ----------------
            xT_sb = cpool.tile([P, 2 * N_DC * TOK_PER_CORE], f16)
            rw_sb = cpool.tile([P, 2 * N_DC * N_COMPRESS], f16)
            kt_sb = cpool.tile([P, N_KNOWLEDGE], f16)
            id_sb = cpool.tile([P, P], f16)
            idf_sb = cpool.tile([P, P], f32)
            iota_sb = cpool.tile([P, 1], f32)
            io16_sb = cpool.tile([P, 24], f32)
            nc.sync.dma_start(out=xT_sb[:], in_=xT[:])
            nc.sync.dma_start(out=rw_sb[:], in_=rw[:])
            nc.sync.dma_start(out=idf_sb[:], in_=idf32[:])
            nc.sync.dma_start(out=id_sb[:], in_=ident[:])
            nc.sync.dma_start(out=iota_sb[:], in_=iotaP[:])
            nc.sync.dma_start(out=io16_sb[:], in_=io16[:])

            wts_sb = cpool.tile([P, N_TILES * N_COMPRESS], f32)

            def tok(t):
                return slice(t * P, (t + 1) * P)

            # ============ A: router softmax (split-bf16 3-pass) ============
            def xsl(hl, dc, t):
                base = hl * N_DC * TOK_PER_CORE
                return xT_sb[:, base + dc * TOK_PER_CORE + t * P:
                             base + dc * TOK_PER_CORE + (t + 1) * P]

            for t in range(N_TILES):
                rps_full = psw.tile([P, 2048], f32, space="PSUM", tag="w", name=f"rps_{t}")
                rps = rps_full[:, :N_COMPRESS]
                NCC = N_COMPRESS
                for dc in range(N_DC):
                    for k, (hl, rl) in enumerate(((0, 0), (0, 1), (1, 0))):
                        nc.tensor.matmul(
                            out=rps, lhsT=xsl(hl, dc, t),
                            rhs=rw_sb[:, (rl * N_DC + dc) * NCC:(rl * N_DC + dc + 1) * NCC],
                            start=(dc == 0 and k == 0), stop=(dc == N_DC - 1 and k == 2))
                w_ = wts_sb[:, t * N_COMPRESS:(t + 1) * N_COMPRESS]
                mx = spool.tile([P, 1], f32, tag="mx")
                sm = spool.tile([P, 1], f32, tag="sm")
                ex = spool.tile([P, N_COMPRESS], f32, tag="ex")
                nc.vector.tensor_reduce(out=mx[:], in_=rps, op=AL.max, axis=mybir.AxisListType.X)
                nc.vector.tensor_scalar(out=ex[:], in0=rps, scalar1=mx[:, :1], scalar2=None,
                                        op0=AL.subtract)
                nc.scalar.activation(out=ex[:], in_=ex[:], func=mybir.ActivationFunctionType.Exp,
                                     accum_out=sm[:, :1])
                rcp = spool.tile([P, 1], f32, tag="rcp")
                nc.vector.reciprocal(out=rcp[:], in_=sm[:, :1])
                nc.vector.tensor_scalar(out=w_, in0=ex[:], scalar1=rcp[:, :1], scalar2=None,
                                        op0=AL.mult)

            # ============ B (per tile-pair): y via split-bf16, Q combine ============
            Q_sb = cpool.tile([P, N_TILES * P], f32)        # exact Q per tile
            TS = {t: {} for t in range(N_TILES)}

            def emit_B(pr):
                tiles = (2 * pr, 2 * pr + 1)
                for gh in range(2):
                    yps = psw.tile([P, 2048], f32, space="PSUM", tag="w", name=f"yps_{pr}_{gh}")
                    for dc in range(N_DC):
                        whb = wgpool.tile([P, 1024], f16, tag="whb", name=f"wh_{pr}_{gh}_{dc}")
                        wlb = wgpool.tile([P, 1024], f16, tag="wlb", name=f"wl_{pr}_{gh}_{dc}")
                        off = (gh * N_DC + dc) * 1024
                        nc.sync.dma_start(out=whb[:], in_=Wg[:, off:off + 1024])
                        nc.sync.dma_start(out=wlb[:], in_=Wg[:, 16384 + off:16384 + off + 1024])
                        for ti, t in enumerate(tiles):
                            for k, (hl, wb) in enumerate(((0, whb), (0, wlb), (1, whb))):
                                for g2_ in range(2):
                                    nc.tensor.matmul(
                                        out=yps[:, ti * 1024 + g2_ * 512:
                                                ti * 1024 + (g2_ + 1) * 512],
                                        lhsT=xsl(hl, dc, t),
                                        rhs=wb[:, g2_ * 512:(g2_ + 1) * 512],
                                        start=(dc == 0 and k == 0),
                                        stop=(dc == N_DC - 1 and k == 2))
                    for ti, t in enumerate(tiles):
                        q_ = Q_sb[:, t * P:(t + 1) * P]
                        for ln in range(8):
                            n = gh * 8 + ln
                            wcol = wts_sb[:, t * N_COMPRESS + n:t * N_COMPRESS + n + 1]
                            ypart = yps[:, ti * 1024 + ln * P:ti * 1024 + (ln + 1) * P]
                            if gh == 0 and ln == 0:
                                nc.vector.tensor_scalar(out=q_, in0=ypart, scalar1=wcol,
                                                        scalar2=None, op0=AL.mult)
                            else:
                                nc.vector.scalar_tensor_tensor(out=q_, in0=ypart, scalar=wcol,
                                                               in1=q_, op0=AL.mult, op1=AL.add)
                # qt16/qf32 for the pair
                for t in tiles:
                    qtps_full = psw.tile([P, 2048], f32, space="PSUM", tag="w", name=f"qtps_{t}")
                    qtps = qtps_full[:, :P]
                    nc.tensor.transpose(out=qtps, in_=Q_sb[:, tok(t)], identity=idf_sb[:])
                    qt16 = qpool.tile([P, P], f16, tag="qt16", name=f"qt16_{t}")
                    nc.scalar.activation(out=qt16[:], in_=qtps,
                                         func=mybir.ActivationFunctionType.Copy,
                                         scale=float(SCALE * SSEL))
                    if dbg:
                        nc.sync.dma_start(out=d_qt[:, tok(t)], in_=qt16[:])
                    qf32 = qpool.tile([P, P], f32, tag="qf32", name=f"qf32_{t}")
                    nc.scalar.activation(out=qf32[:], in_=Q_sb[:, tok(t)],
                                         func=mybir.ActivationFunctionType.Copy,
                                         scale=float(SCALE * SSEL))
                    TS[t].update(qt16=qt16, qf32=qf32)

            def emit_C(t):
                qt16 = TS[t]["qt16"]
                # ============ C: scores + L1 pair-reduce (per half-tile) ============
                bm = dgpool.tile([P, N_BLK], f16, tag="bm", name=f"bm_{t}")
                TS[t]["bm"] = bm
                NBH = N_BLK // 2                         # 128 blocks per half
                for h in range(2):
                    sp1 = sp1pool.tile([P, N_PAIR // 2], f16, tag="sp1", name=f"sp1_{t}_{h}")
                    for wl in range(N_WIN // 2):
                        w = h * (N_WIN // 2) + wl
                        wps = psw.tile([P, WIN], f32, space="PSUM", tag="w", name=f"wps_{t}_{w}")
                        for j in range(4):
                            nc.tensor.matmul(
                                out=wps[:, j * 512:(j + 1) * 512],
                                lhsT=qt16[:],
                                rhs=kt_sb[:, w * WIN + j * 512:w * WIN + (j + 1) * 512],
                                start=True, stop=True)
                        sp1w = sp1[:, wl * HALF:(wl + 1) * HALF]
                        if wl < Z_A // 2:
                            stg = stgpool.tile([P, WIN], f16, tag="stg", name=f"stgA_{t}_{w}")
                            nc.scalar.copy(out=stg[:], in_=wps[:])
                            nc.vector.tensor_tensor(out=sp1w, in0=stg[:, :HALF],
                                                    in1=stg[:, HALF:], op=AL.max)
                        else:
                            stg = stgpool.tile([P, WIN], f16, tag="stg", name=f"stgB_{t}_{w}")
                            nc.scalar.copy(out=stg[:, :HALF], in_=wps[:, HALF:])
                            nc.vector.tensor_tensor(out=sp1w, in0=wps[:, :HALF],
                                                    in1=stg[:, :HALF], op=AL.max)

                    # spill this half: DRAM rows p*256 + h*128 + local
                    o_sp = _copy.copy(sp1D[t][:])
                    o_sp.offset = h * NBH * BLK
                    o_sp.ap = mybir.VecI64Pair([[N_BLK * BLK, P], [1, NBH * BLK]])
                    nc.sync.dma_start(out=o_sp, in_=sp1[:])

                    # tree: blockmax for this half's 128 blocks
                    tw = sp1
                    for wd in (32, 16, 8, 4, 2):
                        tag = "trA" if wd in (32, 8, 2) else "trB"
                        nxt = sp1pool.tile([P, NBH * wd], f16, tag=tag, name=f"tr{wd}_{t}_{h}")
                        s3 = tw[:].rearrange("p (b w) -> p b w", b=NBH)
                        nc.vector.tensor_tensor(out=nxt[:].rearrange("p (b w) -> p b w", b=NBH),
                                                in0=s3[:, :, 0:wd], in1=s3[:, :, wd:2 * wd],
                                                op=AL.max)
                        tw = nxt
                    s3 = tw[:].rearrange("p (b w) -> p b w", b=NBH)
                    nc.vector.tensor_tensor(
                        out=bm[:, h * NBH:(h + 1) * NBH].rearrange("p (b w) -> p b w", b=NBH),
                        in0=s3[:, :, 0:1], in1=s3[:, :, 1:2], op=AL.max)
                if dbg:
                    nc.sync.dma_start(out=d_bm[:, t * N_BLK:(t + 1) * N_BLK], in_=bm[:])

            def emit_tail(t):
                bm = TS[t]["bm"]
                qf32 = TS[t]["qf32"]
                # ============ L2: top-9 blocks (8 + tie slack) ============
                bv8 = spool.tile([P, 8], f16, tag="bv8")
                bu8 = spool.tile([P, 8], u32, tag="bu8")
                nc.vector.max(out=bv8[:], in_=bm[:])
                nc.vector.max_index(out=bu8[:], in_max=bv8[:], in_values=bm[:])
                bmr = spool.tile([P, N_BLK], f16, tag="bmr")
                nc.vector.match_replace(out=bmr[:], in_to_replace=bv8[:], in_values=bm[:],
                                        imm_value=-60000.0)
                bv9 = spool.tile([P, 8], f16, tag="bv9")
                bu9 = spool.tile([P, 8], u32, tag="bu9")
                nc.vector.max(out=bv9[:], in_=bmr[:])
                nc.vector.max_index(out=bu9[:], in_max=bv9[:], in_values=bmr[:])
                bif = spool.tile([P, NBSEL], f32, tag="bif")
                nc.vector.tensor_copy(out=bif[:, :8], in_=bu8[:])
                nc.vector.tensor_copy(out=bif[:, 8:NBSEL], in_=bu9[:, :NBSEL - 8])
                rowb = spool.tile([P, 1], f32, tag="rowb")
                nc.vector.tensor_scalar(out=rowb[:], in0=iota_sb[:], scalar1=float(N_BLK),
                                        scalar2=None, op0=AL.mult)
                gidx = spool.tile([P, NBSEL], f32, tag="gidx")
                nc.vector.tensor_scalar(out=gidx[:], in0=bif[:], scalar1=rowb[:, :1],
                                        scalar2=None, op0=AL.add)
                gidx_u = spool.tile([P, NBSEL], u32, tag="gidx_u")
                nc.vector.tensor_copy(out=gidx_u[:], in_=gidx[:])

                # ============ G1: gather winning blocks ============
                g1 = g1pool.tile([P, NBSEL * BLK], f16, tag="g1", name=f"g1_{t}")
                for s in range(NBSEL):
                    nc.gpsimd.indirect_dma_start(
                        out=g1[:, s * BLK:(s + 1) * BLK], out_offset=None,
                        in_=sp1D[t][:],
                        in_offset=bass.IndirectOffsetOnAxis(ap=gidx_u[:, s:s + 1], axis=0))
                if dbg:
                    nc.sync.dma_start(out=d_g1[:, t * 512:(t + 1) * 512], in_=g1[:, :512])

                # ============ L3: top-10 pairs (8 + tie slack) ============
                pv8 = spool.tile([P, 8], f16, tag="pv8")
                pp8 = spool.tile([P, 8], u32, tag="pp8")
                nc.vector.max(out=pv8[:], in_=g1[:])
                nc.vector.max_index(out=pp8[:], in_max=pv8[:], in_values=g1[:])
                g1r = g1pool.tile([P, NBSEL * BLK], f16, tag="g1r", name=f"g1r_{t}")
                nc.vector.match_replace(out=g1r[:], in_to_replace=pv8[:], in_values=g1[:],
                                        imm_value=-60000.0)
                pv9 = spool.tile([P, 8], f16, tag="pv9")
                pp9 = spool.tile([P, 8], u32, tag="pp9")
                nc.vector.max(out=pv9[:], in_=g1r[:])
                nc.vector.max_index(out=pp9[:], in_max=pv9[:], in_values=g1r[:])
                ppN = spool.tile([P, NPSEL], u32, tag="ppN")
                nc.vector.tensor_copy(out=ppN[:, :8], in_=pp8[:])
                nc.vector.tensor_copy(out=ppN[:, 8:NPSEL], in_=pp9[:, :NPSEL - 8])
                slot_u = spool.tile([P, NPSEL], u32, tag="slot_u")
                nc.vector.tensor_scalar(out=slot_u[:], in0=ppN[:], scalar1=6, scalar2=None,
                                        op0=AL.logical_shift_right)
                slotf = spool.tile([P, NPSEL], f32, tag="slotf")
                ppNf = spool.tile([P, NPSEL], f32, tag="ppNf")
                nc.vector.tensor_copy(out=slotf[:], in_=slot_u[:])
                nc.vector.tensor_copy(out=ppNf[:], in_=ppN[:])
                # off = pp - 64*slot
                offN = spool.tile([P, NPSEL], f32, tag="offN")
                nc.vector.scalar_tensor_tensor(out=offN[:], in0=slotf[:], scalar=-float(BLK),
                                               in1=ppNf[:], op0=AL.mult, op1=AL.add)
                # blk by slot: is_equal-accum over NBSEL slots
                blkj = spool.tile([P, NPSEL], f32, tag="blkj")
                junk = spool.tile([P, NBSEL], f32, tag="junk9")
                for j in range(NPSEL):
                    nc.vector.scalar_tensor_tensor(
                        out=junk[:], in0=io16_sb[:, :NBSEL], scalar=slotf[:, j:j + 1],
                        in1=bif[:], op0=AL.is_equal, op1=AL.mult,
                        accum_out=blkj[:, j:j + 1])
                # pairidx = blk*64 + off
                pj = spool.tile([P, NPSEL], f32, tag="pj")
                nc.vector.scalar_tensor_tensor(out=pj[:], in0=blkj[:], scalar=float(BLK),
                                               in1=offN[:], op0=AL.mult, op1=AL.add)
                if dbg:
                    nc.sync.dma_start(out=d_pj[:, t * NPSEL:(t + 1) * NPSEL], in_=pj[:])
                pj_u = spool.tile([P, NPSEL], u32, tag="pj_u")
                nc.vector.tensor_copy(out=pj_u[:], in_=pj[:])

                # ============ G2 + exact rescore (f32) ============
                g2 = g2pool.tile([P, NPSEL * 2 * RANK], f32, tag="g2", name=f"g2_{t}")
                for s in range(NPSEL):
                    nc.gpsimd.indirect_dma_start(
                        out=g2[:, s * 256:(s + 1) * 256], out_offset=None,
                        in_=KP32[:],
                        in_offset=bass.IndirectOffsetOnAxis(ap=pj_u[:, s:s + 1], axis=0))
                rsc = spool.tile([P, NCAND], f32, tag="rsc")
                HC = NCAND // 2
                for hc in range(2):
                    prod = prpool.tile([P, HC * RANK], f32, tag="prod", name=f"prod_{t}_{hc}")
                    qb = _copy.copy(qf32[:])
                    qb.ap = mybir.VecI64Pair([[qb.ap[0][0], P], [0, HC], [1, RANK]])
                    nc.vector.tensor_tensor(
                        out=prod[:].rearrange("p (c r) -> p c r", c=HC),
                        in0=g2[:, hc * HC * RANK:(hc + 1) * HC * RANK]
                            .rearrange("p (c r) -> p c r", c=HC),
                        in1=qb, op=AL.mult)
                    nc.vector.tensor_reduce(out=rsc[:, hc * HC:(hc + 1) * HC],
                                            in_=prod[:].rearrange("p (c r) -> p c r", c=HC),
                                            op=AL.add, axis=mybir.AxisListType.X)
                if dbg:
                    nc.sync.dma_start(out=d_rs[:, t * NCAND:(t + 1) * NCAND], in_=rsc[:])

                # ============ exact top-8 of NCAND (f32) + keys ============
                v8 = spool.tile([P, 8], f32, tag="v8")
                s8 = spool.tile([P, 8], u32, tag="s8")
                nc.vector.max(out=v8[:], in_=rsc[:])
                nc.vector.max_index(out=s8[:], in_max=v8[:], in_values=rsc[:])
                j8u = spool.tile([P, 8], u32, tag="j8u")
                nc.vector.tensor_scalar(out=j8u[:], in0=s8[:], scalar1=1, scalar2=None,
                                        op0=AL.logical_shift_right)
                j8f = spool.tile([P, 8], f32, tag="j8f")
                s8f = spool.tile([P, 8], f32, tag="s8f")
                nc.vector.tensor_copy(out=j8f[:], in_=j8u[:])
                nc.vector.tensor_copy(out=s8f[:], in_=s8[:])
                m8 = spool.tile([P, 8], f32, tag="m8")      # member = s - 2*j
                nc.vector.scalar_tensor_tensor(out=m8[:], in0=j8f[:], scalar=-2.0,
                                               in1=s8f[:], op0=AL.mult, op1=AL.add)
                # pairidx by j
                psel = spool.tile([P, 8], f32, tag="psel")
                junk2 = spool.tile([P, NPSEL], f32, tag="junk10")
                for j in range(8):
                    nc.vector.scalar_tensor_tensor(
                        out=junk2[:], in0=io16_sb[:, :NPSEL], scalar=j8f[:, j:j + 1], in1=pj[:],
                        op0=AL.is_equal, op1=AL.mult, accum_out=psel[:, j:j + 1])
                psel_u = spool.tile([P, 8], u32, tag="psel_u")
                wsel_u = spool.tile([P, 8], u32, tag="wsel_u")
                nc.vector.tensor_copy(out=psel_u[:], in_=psel[:])
                nc.vector.tensor_scalar(out=wsel_u[:], in0=psel_u[:], scalar1=10, scalar2=None,
                                        op0=AL.logical_shift_right)
                wself = spool.tile([P, 8], f32, tag="wself")
                nc.vector.tensor_copy(out=wself[:], in_=wsel_u[:])
                # key = psel + 1024*wsel + 1024*m
                keyf = spool.tile([P, 8], f32, tag="keyf")
                nc.vector.scalar_tensor_tensor(out=keyf[:], in0=wself[:], scalar=1024.0,
                                               in1=psel[:], op0=AL.mult, op1=AL.add)
                nc.vector.scalar_tensor_tensor(out=keyf[:], in0=m8[:], scalar=1024.0,
                                               in1=keyf[:], op0=AL.mult, op1=AL.add)
                if dbg:
                    nc.sync.dma_start(out=d_key[:, t * 8:(t + 1) * 8], in_=keyf[:])
                key_u = spool.tile([P, 8], u32, tag="key_u")
                nc.vector.tensor_copy(out=key_u[:], in_=keyf[:])

                # softmax over v8 (descending, v8[0] is max); exp scale 1/SSEL
                w8 = spool.tile([P, 8], f32, tag="w8")
                sm8 = spool.tile([P, 1], f32, tag="sm8")
                nc.vector.tensor_scalar(out=w8[:], in0=v8[:], scalar1=v8[:, :1], scalar2=None,
                                        op0=AL.subtract)
                nc.scalar.activation(out=w8[:], in_=w8[:], func=mybir.ActivationFunctionType.Exp,
                                     scale=float(1.0 / SSEL), accum_out=sm8[:, :1])
                rcp8 = spool.tile([P, 1], f32, tag="rcp8")
                nc.vector.reciprocal(out=rcp8[:], in_=sm8[:, :1])
                nc.vector.tensor_scalar(out=w8[:], in0=w8[:], scalar1=rcp8[:, :1], scalar2=None,
                                        op0=AL.mult)
                if dbg:
                    nc.sync.dma_start(out=d_w8[:, t * 8:(t + 1) * 8], in_=w8[:])

                # ============ G3 + Vsum via diag matmuls ============
                g3 = g3pool.tile([P, 8 * D_MODEL], f16, tag="g3", name=f"g3_{t}")
                for s in range(8):
                    nc.gpsimd.indirect_dma_start(
                        out=g3[:, s * D_MODEL:(s + 1) * D_MODEL], out_offset=None,
                        in_=VD16[:],
                        in_offset=bass.IndirectOffsetOnAxis(ap=key_u[:, s:s + 1], axis=0))
                dg8 = dgpool.tile([P, 8 * P], f16, tag="dg8", name=f"dg8_{t}")
                for s in range(8):
                    nc.vector.tensor_scalar(out=dg8[:, s * P:(s + 1) * P], in0=id_sb[:],
                                            scalar1=w8[:, s:s + 1], scalar2=None, op0=AL.mult)
                accps_full = psw.tile([P, 2048], f32, space="PSUM", tag="w", name=f"accps_{t}")
                accps = accps_full[:, :D_MODEL]
                for h in range(2):
                    for s in range(8):
                        nc.tensor.matmul(
                            out=accps[:, h * 512:(h + 1) * 512],
                            lhsT=dg8[:, s * P:(s + 1) * P],
                            rhs=g3[:, s * D_MODEL + h * 512:s * D_MODEL + (h + 1) * 512],
                            start=(s == 0), stop=(s == 7))
                accf = apool.tile([P, D_MODEL], f32, tag="accf", name=f"accf_{t}")
                nc.scalar.copy(out=accf[:], in_=accps)
                nc.sync.dma_start(out=out[t * P:(t + 1) * P, :], in_=accf[:])
                if dbg:
                    nc.sync.dma_start(out=d_wts[:], in_=wts_sb[:])

            emit_B(0)
            # KT16 loads issued after B0's Wg stream so they don't block it
            for q in range(4):
                nc.sync.dma_start(out=kt_sb[:, q * 8192:(q + 1) * 8192],
                                  in_=KT16[:, q * 8192:(q + 1) * 8192])
            emit_C(0)
            emit_B(1)
            emit_C(1)
            emit_tail(0)
            emit_C(2)
            emit_tail(1)
            emit_C(3)
            emit_tail(2)
            emit_tail(3)

    nc.compile()
    return nc


_NC_CACHE = {}


def _get_nc(dbg=False):
    if dbg not in _NC_CACHE:
        _NC_CACHE[dbg] = _build(dbg)
    return _NC_CACHE[dbg]


def _split16(a):
    hi = a.astype(np.float16)
    lo = (a - hi.astype(np.float32)).astype(np.float16)
    return hi, lo


def _prep_in_maps(x, router_w, compress_neurons, knowledge_K, knowledge_V):
    x = np.asarray(x, dtype=np.float32).reshape(B * S, D_MODEL)
    rwT = np.asarray(router_w, dtype=np.float32).T          # [1024, 16]
    rw_f = np.ascontiguousarray(
        rwT.reshape(N_DC, P, N_COMPRESS).transpose(1, 0, 2).reshape(P, N_DC * N_COMPRESS))
    rw_r = np.concatenate(_split16(rw_f), axis=1)           # [P, 2*8*16]
    cn = np.asarray(compress_neurons, dtype=np.float32)     # [16, 1024, 128]
    # Wg[p, hl*16384 + (gh*8+dc)*1024 + ln*128 + r] = cn_hl[8*gh+ln, dc*128+p, r]
    Wg_f = np.ascontiguousarray(
        cn.reshape(2, 8, N_DC, P, RANK).transpose(3, 0, 2, 1, 4).reshape(P, 2 * N_DC * 1024))
    Wg = np.concatenate(_split16(Wg_f), axis=1)             # [P, 32768]
    K = np.asarray(knowledge_K, dtype=np.float32)
    KT16 = np.ascontiguousarray(K.T).astype(np.float16)     # [128, 32768]
    # KP32[w*1024+j] = (K[2048w+j], K[2048w+1024+j]) in f32
    KP32 = np.ascontiguousarray(
        K.reshape(N_WIN, 2, HALF, RANK).transpose(0, 2, 1, 3).reshape(N_PAIR, 2 * RANK))
    V16 = np.asarray(knowledge_V, dtype=np.float32).astype(np.float16)
    ident = np.eye(P, dtype=np.float16)
    idf32 = np.eye(P, dtype=np.float32)
    iotaP = np.arange(P, dtype=np.float32).reshape(P, 1)
    io16 = np.broadcast_to(np.arange(24, dtype=np.float32), (P, 24)).copy()

    in_maps = []
    for c in range(N_CORES):
        xs = x[c * TOK_PER_CORE:(c + 1) * TOK_PER_CORE]
        xTf = np.ascontiguousarray(
            xs.T.reshape(N_DC, P, TOK_PER_CORE).transpose(1, 0, 2).reshape(P, N_DC * TOK_PER_CORE))
        xTc = np.concatenate(_split16(xTf), axis=1)
        in_maps.append(dict(xT=xTc, rw=rw_r, Wg=Wg, KT16=KT16, KP32=KP32, VD16=V16,
                            ident=ident, idf32=idf32, iotaP=iotaP, io16=io16))
    return in_maps


def _ensure_ntff_hook():
    import sys as _sys
    import types as _types
    if "antenv.axon_hooks" in _sys.modules:
        return
    try:
        import antenv.axon_hooks  # noqa: F401
        return
    except ImportError:
        pass
    mod = _types.ModuleType("antenv.axon_hooks")
    _state = {"hook": None}
    mod.set_axon_ntff_profile_hook = lambda h: _state.__setitem__("hook", h)
    mod.get_axon_ntff_profile_hook = lambda: _state["hook"]
    _sys.modules["antenv.axon_hooks"] = mod
    try:
        from trn_agent_boot.trn_boot import _ntff_profile_via_ctypes
        mod.set_axon_ntff_profile_hook(_ntff_profile_via_ctypes("/opt/axon/libaxon_pjrt.so"))
    except Exception:
        pass


def _run(inputs, trace=False, dbg=False):
    if trace:
        _ensure_ntff_hook()
    nc = _get_nc(dbg)
    in_maps = _prep_in_maps(**inputs)
    res = run_bass_kernel_spmd(nc, in_maps, core_ids=list(range(N_CORES)), trace=trace)
    out = np.concatenate([res.results[c]["out"] for c in range(N_CORES)], axis=0)
    return out.reshape(B, S, D_MODEL).astype(np.float32), res


def kernel(x, router_w, compress_neurons, knowledge_K, knowledge_V):
    out, _ = _run(dict(x=x, router_w=router_w, compress_neurons=compress_neurons,
                       knowledge_K=knowledge_K, knowledge_V=knowledge_V))
    return out
